# revision 24
# baseline (speedup 1.0000x reference)
"""KAN Fourier-linear kernel for 8 Trainium2 NeuronCores.

y[n,o] = sum_{i,g} C0[o,i,g]*cos(g*x[n,i]) + C1[o,i,g]*sin(g*x[n,i]) + bias[o]

Strategy (data-parallel over n, 4096 rows/core), double-angle cascade:
  - ACT (ScalarE) computes sin/cos only for odd g in {1,3,...,15} (16 Sin
    passes per superpass instead of 64), via the int-round range reduction:
      v   = int32(x*g/2pi + magic)        # gpsimd tensor_scalar
      r_g = x - v*(2pi/g)                 # DVE scalar_tensor_tensor (fp32)
      s_g = Sin(scale=g, bias=b_s)(r_g); c_g = Sin(scale=g, bias=b_c)(r_g)
  - Even harmonics come from 1-op DVE fp16 products with the scale factors
    folded into the weights host-side:
      u_m = s_m*c_m   covers sin(2m x) = kappa_m * u_m
      v_m = s_m*s_m   covers cos(2m x) = 1 - lambda_m * v_m
    (constants fold into the output bias). True-cos intermediates C_{2m} =
    1 - lambda*v_m (one tensor_scalar) extend the cascade to m in {2,4,6,8}.
  - Everything in fp16 (not bf16): the x128 weight folds amplify feature
    rounding error; fp16's 10-bit mantissa keeps rel err ~2e-3.
  - y.T tile = W.T @ F via PE, K=4096 accumulated in PSUM (fp16 inputs).
  - PSUM evicted by ACT Copy with fused per-partition bias add, fp16 out.
"""
import math
import numpy as np
from contextlib import ExitStack

import concourse.bass as bass
import concourse.mybir as mybir
import concourse.tile as tile
from concourse import bacc
from concourse.bass_utils import run_bass_kernel_spmd

N_CORES = 8
N_TOTAL = 32768
N_SHARD = N_TOTAL // N_CORES        # 4096 rows per core
INDIM = 128
OUTDIM = 256
GRID = 16
K_TOT = 2 * GRID * INDIM            # 4096
# superpass column widths: narrow passes up front so the first
# feature chain (and hence the first matmul) starts early, wide passes
# for the bulk to keep per-instruction overhead low.
SP_W = (256, 256, 512, 1024, 1024, 1024)
SP = len(SP_W)
SP_OFF = tuple(int(np.sum(SP_W[:i])) for i in range(SP + 1))
CH = 512                            # matmul moving chunk (PSUM bank limit)
TWO_PI = 2.0 * math.pi

FP32 = mybir.dt.float32
FP16 = mybir.dt.float16
I32 = mybir.dt.int32

ODD = (1, 3, 5, 7, 9, 11, 13, 15)
# kt consumption order: feature name per contraction block.
# Interleaved so production (ACT for s/c, DVE for u/v) stays ahead of the PE.
KT_ORDER = [
    ("s", 1), ("c", 1), ("u", 1), ("v", 1),
    ("s", 3), ("c", 3), ("u", 3), ("v", 3),
    ("s", 5), ("c", 5), ("u", 5), ("v", 5),
    ("s", 7), ("c", 7), ("u", 7), ("v", 7),
    ("s", 9), ("c", 9), ("u", 2), ("v", 2),
    ("s", 11), ("c", 11), ("u", 4), ("v", 4),
    ("s", 13), ("c", 13), ("u", 6), ("v", 6),
    ("s", 15), ("c", 15), ("u", 8), ("v", 8),
]
KAPPA = {1: 2.0, 2: 4.0, 3: 2.0, 4: 8.0, 5: 2.0, 6: 4.0, 7: 2.0, 8: 16.0}
LAMBDA = {1: 2.0, 2: 8.0, 3: 2.0, 4: 32.0, 5: 2.0, 6: 8.0, 7: 2.0, 8: 128.0}


def _g_consts(g: int):
    a = np.float32(g / TWO_PI)
    phat = np.float32(TWO_PI / g)
    m = 2.0 ** math.ceil(math.log2(0.960 * g + 0.14))
    c = np.float32(m + 0.125)
    b_s = np.float32(m * g * float(phat))      # == 2pi*m up to fp32, matched to phat
    b_c = np.float32(float(b_s) + math.pi / 2.0)
    return a, phat, c, b_s, b_c


_CACHED = {}


def _build(reps: int = 1):
    key = ("nc", reps)
    if key in _CACHED:
        return _CACHED[key]
    nc = bacc.Bacc("TRN2", target_bir_lowering=False, debug=False,
                   num_devices=N_CORES)
    xt_d = nc.dram_tensor("xt", [INDIM, N_SHARD], FP32, kind="ExternalInput").ap()
    w_d = nc.dram_tensor("w", [INDIM, 32 * OUTDIM], FP16, kind="ExternalInput").ap()
    btb_d = nc.dram_tensor("btb", [INDIM, 18], FP32, kind="ExternalInput").ap()
    yt_d = nc.dram_tensor("yt", [OUTDIM, N_SHARD], FP16, kind="ExternalOutput").ap()

    with tile.TileContext(nc) as tc, ExitStack() as ctx:
        cpool = ctx.enter_context(tc.tile_pool(name="const", bufs=1))
        vpool = ctx.enter_context(tc.tile_pool(name="v", bufs=6))
        rpool = ctx.enter_context(tc.tile_pool(name="r", bufs=4))
        apool = ctx.enter_context(tc.tile_pool(name="af", bufs=16))
        dpool = ctx.enter_context(tc.tile_pool(name="df", bufs=2))
        ypool = ctx.enter_context(tc.tile_pool(name="y", bufs=2))
        ppool = ctx.enter_context(tc.tile_pool(name="psum", bufs=2, space="PSUM"))

        # DMA priority order: the first matmul needs (a) the first weight
        # block and (b) features derived from x[sp0].  Land those first,
        # then interleave the rest so neither stream starves.
        xt = cpool.tile([INDIM, N_SHARD], FP32)
        wt = cpool.tile([INDIM, 32 * OUTDIM], FP16)
        btb = cpool.tile([INDIM, 18], FP32)
        bt = btb[:, 0:16]
        bias = btb[:, 16:18]
        # x chunk 0 first (it gates the whole feature chain), then the
        # ACT bias table, first weight block, and the rest interleaved.
        nc.sync.dma_start(xt[:, 0:256], xt_d[:, 0:256])
        nc.sync.dma_start(btb[:], btb_d[:])
        nc.sync.dma_start(wt[:, 0:2 * OUTDIM], w_d[:, 0:2 * OUTDIM])
        x_chunks = [(256, 512), (512, 1536), (1536, 2560), (2560, 3584),
                    (3584, 4096)]
        w_slabs = [(2, 8), (8, 14), (14, 20), (20, 26), (26, 32)]
        for (xlo, xhi), (klo, khi) in zip(x_chunks, w_slabs):
            nc.sync.dma_start(xt[:, xlo:xhi], xt_d[:, xlo:xhi])
            nc.sync.dma_start(wt[:, klo * OUTDIM:khi * OUTDIM],
                              w_d[:, klo * OUTDIM:khi * OUTDIM])
        xts = [xt[:, SP_OFF[sp]:SP_OFF[sp + 1]] for sp in range(SP)]

        # view of yt for single-descriptor stores covering both row halves:
        # [r, h, c] -> yt[h*128 + r, c]
        yt_r = yt_d.rearrange("(h r) n -> r h n", h=2)

        def emit_evict(psums, sp, lo=0, hi=None):
            # split evictions across DVE (oh0) and ACT (oh1) so neither
            # queue eats both PSUM reads; store both halves with ONE DMA
            # descriptor (HWDGE descriptor processing is ~625ns, serial).
            # [lo,hi) selects a column chunk of the superpass.
            if hi is None:
                hi = SP_W[sp]
            osl = slice(SP_OFF[sp] + lo, SP_OFF[sp] + hi)
            y01 = ypool.tile([128, 2, 1024], FP16, tag="y01")
            nc.vector.tensor_scalar(y01[:, 0, lo:hi], psums[0][:, lo:hi],
                                    bias[:, 0:1], None,
                                    mybir.AluOpType.add)
            nc.scalar.activation(y01[:, 1, lo:hi], psums[1][:, lo:hi],
                                 mybir.ActivationFunctionType.Identity,
                                 bias=bias[:, 1:2], scale=1.0)
            nc.sync.dma_start(yt_r[:, :, osl], y01[:, :, lo:hi])

        def body():
            pending = None
            for sp in range(SP):
                xs = xts[sp]
                W = SP_W[sp]
                feats = {}

                # feature tiles are allocated at the max width (1024) so a
                # pool tag always recycles equal-size buffers; narrow
                # superpasses just use the first W columns.

                # -- range reduction + ACT passes for odd g --
                def reduce_g(g, j):
                    a, phat, c, b_s, b_c = _g_consts(g)
                    v = vpool.tile([INDIM, 1024], I32, tag="v")
                    nc.gpsimd.tensor_scalar(v[:, 0:W], xs, float(a), float(c),
                                            mybir.AluOpType.mult,
                                            mybir.AluOpType.add)
                    r = rpool.tile([INDIM, 1024], FP32, tag="r")
                    nc.vector.scalar_tensor_tensor(r[:, 0:W], v[:, 0:W],
                                                   float(-phat), xs,
                                                   mybir.AluOpType.mult,
                                                   mybir.AluOpType.add)
                    s = apool.tile([INDIM, 1024], FP16, tag="act")
                    nc.scalar.activation(s[:, 0:W], r[:, 0:W],
                                         mybir.ActivationFunctionType.Sin,
                                         bias=bt[:, 2 * j:2 * j + 1],
                                         scale=float(g))
                    cc = apool.tile([INDIM, 1024], FP16, tag="act")
                    nc.scalar.activation(cc[:, 0:W], r[:, 0:W],
                                         mybir.ActivationFunctionType.Sin,
                                         bias=bt[:, 2 * j + 1:2 * j + 2],
                                         scale=float(g))
                    feats[("s", g)] = s
                    feats[("c", g)] = cc

                def prod(name, m, in0, in1):
                    t = dpool.tile([INDIM, 1024], FP16, tag=f"{name}{m}")
                    nc.vector.tensor_tensor(t[:, 0:W], in0[:, 0:W],
                                            in1[:, 0:W],
                                            mybir.AluOpType.mult)
                    feats[(name, m)] = t
                    return t

                def truecos(m, vm, lam):
                    t = dpool.tile([INDIM, 1024], FP16, tag=f"C{2*m}")
                    nc.vector.tensor_scalar(t[:, 0:W], vm[:, 0:W],
                                            float(-lam), 1.0,
                                            mybir.AluOpType.mult,
                                            mybir.AluOpType.add)
                    return t

                # emission order = per-engine program order; keep DVE stream
                # interleaved so fmas feed ACT early and cascade flows.
                reduce_g(1, 0)
                reduce_g(3, 1)
                u1 = prod("u", 1, feats[("s", 1)], feats[("c", 1)])
                v1 = prod("v", 1, feats[("s", 1)], feats[("s", 1)])
                C2 = truecos(1, v1, LAMBDA[1])
                reduce_g(5, 2)
                u3 = prod("u", 3, feats[("s", 3)], feats[("c", 3)])
                v3 = prod("v", 3, feats[("s", 3)], feats[("s", 3)])
                C6 = truecos(3, v3, LAMBDA[3])
                reduce_g(7, 3)
                u5 = prod("u", 5, feats[("s", 5)], feats[("c", 5)])
                v5 = prod("v", 5, feats[("s", 5)], feats[("s", 5)])
                reduce_g(9, 4)
                u7 = prod("u", 7, feats[("s", 7)], feats[("c", 7)])
                v7 = prod("v", 7, feats[("s", 7)], feats[("s", 7)])
                reduce_g(11, 5)
                u2 = prod("u", 2, u1, C2)
                v2 = prod("v", 2, u1, u1)
                C4 = truecos(2, v2, LAMBDA[2])
                reduce_g(13, 6)
                u4 = prod("u", 4, u2, C4)
                v4 = prod("v", 4, u2, u2)
                C8 = truecos(4, v4, LAMBDA[4])
                reduce_g(15, 7)
                u6 = prod("u", 6, u3, C6)
                v6 = prod("v", 6, u3, u3)
                u8 = prod("u", 8, u4, C8)
                v8 = prod("v", 8, u4, u4)

                # previous superpass's eviction goes AFTER this superpass's
                # feature emission so ACT's program order is
                # [sp passes][sp+1 passes][evict sp] — eviction fires the
                # moment the PE frees the PSUM, without blocking features.
                if pending is not None:
                    emit_evict(*pending)

                # -- matmuls --
                n_kt = len(KT_ORDER)
                psum0 = ppool.tile([128, 1024], FP32, tag="p0")
                psum1 = ppool.tile([128, 1024], FP32, tag="p1")
                psums = [psum0, psum1]
                last = sp == SP - 1
                if last and pending is not None:
                    emit_evict(*pending)
                    pending = None
                # last superpass runs its two 512-column PSUM chunks as
                # sequential accumulation groups (chi-outer) so chunk 0's
                # eviction + store overlap chunk 1's matmuls -> short tail.
                # Other superpasses interleave chunks inside the kt loop.
                all_chunks = [(c, min(c + CH, W)) for c in range(0, W, CH)]
                chunk_groups = [[ck] for ck in all_chunks] if last \
                    else [all_chunks]
                for grp in chunk_groups:
                    for kt, fkey in enumerate(KT_ORDER):
                        f = feats[fkey]
                        for oh in range(2):
                            lhsT = wt[:, kt * OUTDIM + oh * 128:
                                      kt * OUTDIM + oh * 128 + 128]
                            for (clo, chi_) in grp:
                                nc.tensor.matmul(
                                    psums[oh][:, clo:chi_],
                                    lhsT, f[:, clo:chi_],
                                    start=(kt == 0), stop=(kt == n_kt - 1),
                                )
                    if last:
                        (clo, chi_), = grp
                        emit_evict(psums, sp, lo=clo, hi=chi_)
                if not last:
                    pending = (psums, sp)

        if reps == 1:
            body()
        else:
            with tc.For_i(0, reps, 1):
                body()

    nc.compile()
    _CACHED[key] = nc
    return nc


def _prep_inputs(x: np.ndarray, fouriercoeffs: np.ndarray, bias: np.ndarray):
    xt = np.ascontiguousarray(x.astype(np.float32, copy=False).T)  # (128, 32768)
    C0 = fouriercoeffs[0].astype(np.float32)   # (256, 128, 16) cos coeffs
    C1 = fouriercoeffs[1].astype(np.float32)   # sin coeffs

    # folded weight blocks per kt: w_sb[i, kt*256 + col], col = output o
    w_sb = np.empty((INDIM, 32 * OUTDIM), np.float32)
    for kt, (name, m) in enumerate(KT_ORDER):
        if name == "s":
            blk = C1[:, :, m - 1]                      # (o, i)
        elif name == "c":
            blk = C0[:, :, m - 1]
        elif name == "u":
            blk = KAPPA[m] * C1[:, :, 2 * m - 1]
        else:  # "v"
            blk = -LAMBDA[m] * C0[:, :, 2 * m - 1]
        w_sb[:, kt * OUTDIM:(kt + 1) * OUTDIM] = blk.T
    w_sb = w_sb.astype(np.float16)

    # ACT bias table: 16 passes in order (b_s, b_c) per odd g
    bvals = np.empty(16, np.float32)
    for j, g in enumerate(ODD):
        _, _, _, b_s, b_c = _g_consts(g)
        bvals[2 * j] = b_s
        bvals[2 * j + 1] = b_c
    bt = np.tile(bvals[None, :], (INDIM, 1)).astype(np.float32)

    # folded output bias: bias + sum_i C0[o,i,2m-1] over even harmonics
    bias_fold = bias.reshape(-1).astype(np.float64).copy()
    for m in (1, 2, 3, 4, 5, 6, 7, 8):
        bias_fold += C0[:, :, 2 * m - 1].astype(np.float64).sum(axis=1)
    bias_sb = np.ascontiguousarray(
        bias_fold.astype(np.float32).reshape(2, 128).T)      # (128, 2)
    btb = np.ascontiguousarray(
        np.concatenate([bt, bias_sb], axis=1))               # (128, 18)
    return xt, w_sb, btb


def kernel(x: np.ndarray, fouriercoeffs: np.ndarray, bias: np.ndarray,
           _trace: bool = False):
    x = np.asarray(x)
    fouriercoeffs = np.asarray(fouriercoeffs)
    bias = np.asarray(bias)
    orig_shape = x.shape
    x2 = x.reshape(-1, INDIM)
    assert x2.shape == (N_TOTAL, INDIM), x2.shape

    nc = _build()
    xt, w_sb, btb = _prep_inputs(x2, fouriercoeffs, bias)
    in_maps = []
    for c in range(N_CORES):
        in_maps.append({
            "xt": np.ascontiguousarray(xt[:, c * N_SHARD:(c + 1) * N_SHARD]),
            "w": w_sb,
            "btb": btb,
        })
    res = run_bass_kernel_spmd(nc, in_maps, list(range(N_CORES)),
                               trace=_trace)
    yt = np.concatenate([res.results[c]["yt"] for c in range(N_CORES)], axis=1)
    y = np.ascontiguousarray(yt.T).astype(np.float32)
    if _trace:
        kernel._last_result = res
    return y.reshape(*orig_shape[:-1], OUTDIM)



# revision 34
# speedup vs baseline: 1.0613x; 1.0613x over previous
"""KAN Fourier-linear kernel for 8 Trainium2 NeuronCores.

y[n,o] = sum_{i,g} C0[o,i,g]*cos(g*x[n,i]) + C1[o,i,g]*sin(g*x[n,i]) + bias[o]

Strategy (data-parallel over n, 4096 rows/core), double-angle cascade:
  - ACT (ScalarE) computes sin/cos only for odd g in {1,3,...,15} (16 Sin
    passes per superpass instead of 64), via the int-round range reduction:
      v   = int32(x*g/2pi + magic)        # gpsimd tensor_scalar
      r_g = x - v*(2pi/g)                 # DVE scalar_tensor_tensor (fp32)
      s_g = Sin(scale=g, bias=b_s)(r_g); c_g = Sin(scale=g, bias=b_c)(r_g)
  - Even harmonics come from 1-op DVE fp16 products with the scale factors
    folded into the weights host-side:
      u_m = s_m*c_m   covers sin(2m x) = kappa_m * u_m
      v_m = s_m*s_m   covers cos(2m x) = 1 - lambda_m * v_m
    (constants fold into the output bias). True-cos intermediates C_{2m} =
    1 - lambda*v_m (one tensor_scalar) extend the cascade to m in {2,4,6,8}.
  - Everything in fp16 (not bf16): the x128 weight folds amplify feature
    rounding error; fp16's 10-bit mantissa keeps rel err ~2e-3.
  - y.T tile = W.T @ F via PE, K=4096 accumulated in PSUM (fp16 inputs).
  - PSUM evicted by ACT Copy with fused per-partition bias add, fp16 out.
"""
import math
import numpy as np
from contextlib import ExitStack

import concourse.bass as bass
import concourse.mybir as mybir
import concourse.tile as tile
from concourse import bacc
from concourse.bass_utils import run_bass_kernel_spmd

N_CORES = 8
N_TOTAL = 32768
N_SHARD = N_TOTAL // N_CORES        # 4096 rows per core
INDIM = 128
OUTDIM = 256
GRID = 16
K_TOT = 2 * GRID * INDIM            # 4096
# superpass column widths: two narrow passes up front so the first
# feature chain (and hence the first matmul) starts early, wide passes
# for the bulk to keep per-instruction overhead low.
SP_W = (512, 512, 1024, 1024, 1024)
SP = len(SP_W)
SP_OFF = tuple(int(np.sum(SP_W[:i])) for i in range(SP + 1))
CH = 512                            # matmul moving chunk (PSUM bank limit)
TWO_PI = 2.0 * math.pi

FP32 = mybir.dt.float32
FP16 = mybir.dt.float16
FP8 = mybir.dt.float8e4
I32 = mybir.dt.int32

ODD = (1, 3, 5, 7, 9, 11, 13, 15)
# Harmonics whose (sin, cos) pair is packed into one fp8 tile and
# contracted with a single DoubleRow matmul (2 K-tiles per stream pass,
# 2x MAC rate).  4 of 32 K-blocks in fp8 keeps the extra quantization
# error at ~1.4e-2 max-rel (tolerance 2e-2); measured DR throughput is
# ~291 ns per N=512 DR-MM == the plain fp16 rate at twice the MACs.
DR_G = (9, 11)
# kt consumption order: feature name per contraction block ("dr", g)
# entries consume a packed fp8 pair via one DoubleRow matmul.
# Interleaved so production (ACT for s/c, DVE for u/v) stays ahead of the PE.
KT_ORDER = [
    ("s", 1), ("c", 1), ("u", 1), ("v", 1),
    ("s", 3), ("c", 3), ("u", 3), ("v", 3),
    ("s", 5), ("c", 5), ("u", 5), ("v", 5),
    ("s", 7), ("c", 7), ("u", 7), ("v", 7),
    ("dr", 9), ("u", 2), ("v", 2),
    ("dr", 11), ("u", 4), ("v", 4),
    ("s", 13), ("c", 13), ("u", 6), ("v", 6),
    ("s", 15), ("c", 15), ("u", 8), ("v", 8),
]
# fp16 weight-block index for each non-dr entry, in order
F16_IDX = {}
for _e in KT_ORDER:
    if _e[0] != "dr":
        F16_IDX[_e] = len(F16_IDX)
N_F16 = len(F16_IDX)                # 28 fp16 K-blocks
KAPPA = {1: 2.0, 2: 4.0, 3: 2.0, 4: 8.0, 5: 2.0, 6: 4.0, 7: 2.0, 8: 16.0}
LAMBDA = {1: 2.0, 2: 8.0, 3: 2.0, 4: 32.0, 5: 2.0, 6: 8.0, 7: 2.0, 8: 128.0}


def _g_consts(g: int):
    a = np.float32(g / TWO_PI)
    phat = np.float32(TWO_PI / g)
    m = 2.0 ** math.ceil(math.log2(0.960 * g + 0.14))
    c = np.float32(m + 0.125)
    b_s = np.float32(m * g * float(phat))      # == 2pi*m up to fp32, matched to phat
    b_c = np.float32(float(b_s) + math.pi / 2.0)
    return a, phat, c, b_s, b_c


_CACHED = {}


def _build(reps: int = 1):
    key = ("nc", reps)
    if key in _CACHED:
        return _CACHED[key]
    nc = bacc.Bacc("TRN2", target_bir_lowering=False, debug=False,
                   num_devices=N_CORES)
    xt_d = nc.dram_tensor("xt", [INDIM, N_SHARD], FP32, kind="ExternalInput").ap()
    w_d = nc.dram_tensor("w", [INDIM, N_F16 * OUTDIM], FP16,
                         kind="ExternalInput").ap()
    w8_d = nc.dram_tensor("w8", [INDIM, 2, 2 * OUTDIM], FP8,
                          kind="ExternalInput").ap()
    btb_d = nc.dram_tensor("btb", [INDIM, 18], FP32, kind="ExternalInput").ap()
    yt_d = nc.dram_tensor("yt", [OUTDIM, N_SHARD], FP16, kind="ExternalOutput").ap()

    with tile.TileContext(nc) as tc, ExitStack() as ctx:
        cpool = ctx.enter_context(tc.tile_pool(name="const", bufs=1))
        vpool = ctx.enter_context(tc.tile_pool(name="v", bufs=6))
        rpool = ctx.enter_context(tc.tile_pool(name="r", bufs=4))
        apool = ctx.enter_context(tc.tile_pool(name="af", bufs=16))
        f8pool = ctx.enter_context(tc.tile_pool(name="f8", bufs=2))
        dpool = ctx.enter_context(tc.tile_pool(name="df", bufs=2))
        ypool = ctx.enter_context(tc.tile_pool(name="y", bufs=2))
        ppool = ctx.enter_context(tc.tile_pool(name="psum", bufs=2, space="PSUM"))

        # DMA priority order: the first matmul needs (a) the first weight
        # block and (b) features derived from x[sp0].  Land those first,
        # then interleave the rest so neither stream starves.
        xt = cpool.tile([INDIM, N_SHARD], FP32)
        wt = cpool.tile([INDIM, N_F16 * OUTDIM], FP16)
        w8t = cpool.tile([INDIM, 2, 2 * OUTDIM], FP8)
        btb = cpool.tile([INDIM, 18], FP32)
        bt = btb[:, 0:16]
        bias = btb[:, 16:18]
        # x chunk 0 first (it gates the whole feature chain), then the
        # ACT bias table, first weight block, and the rest interleaved.
        nc.sync.dma_start(xt[:, 0:512], xt_d[:, 0:512])
        nc.sync.dma_start(btb[:], btb_d[:])
        nc.sync.dma_start(wt[:, 0:2 * OUTDIM], w_d[:, 0:2 * OUTDIM])
        x_chunks = [(512, 1536), (1536, 2560), (2560, 3584), (3584, 4096)]
        w_slabs = [(2, 9), (9, 16), (16, 22), (22, 28)]
        for ci, ((xlo, xhi), (klo, khi)) in enumerate(zip(x_chunks, w_slabs)):
            nc.sync.dma_start(xt[:, xlo:xhi], xt_d[:, xlo:xhi])
            nc.sync.dma_start(wt[:, klo * OUTDIM:khi * OUTDIM],
                              w_d[:, klo * OUTDIM:khi * OUTDIM])
            if ci == 0:
                nc.sync.dma_start(w8t[:], w8_d[:])
        xts = [xt[:, SP_OFF[sp]:SP_OFF[sp + 1]] for sp in range(SP)]

        # view of yt for single-descriptor stores covering both row halves:
        # [r, h, c] -> yt[h*128 + r, c]
        yt_r = yt_d.rearrange("(h r) n -> r h n", h=2)

        def emit_evict(psums, sp, lo=0, hi=None):
            # split evictions across DVE (oh0) and ACT (oh1) so neither
            # queue eats both PSUM reads; store both halves with ONE DMA
            # descriptor (HWDGE descriptor processing is ~625ns, serial).
            # [lo,hi) selects a column chunk of the superpass.
            if hi is None:
                hi = SP_W[sp]
            osl = slice(SP_OFF[sp] + lo, SP_OFF[sp] + hi)
            y01 = ypool.tile([128, 2, 1024], FP16, tag="y01")
            nc.vector.tensor_scalar(y01[:, 0, lo:hi], psums[0][:, lo:hi],
                                    bias[:, 0:1], None,
                                    mybir.AluOpType.add)
            nc.scalar.activation(y01[:, 1, lo:hi], psums[1][:, lo:hi],
                                 mybir.ActivationFunctionType.Identity,
                                 bias=bias[:, 1:2], scale=1.0)
            nc.sync.dma_start(yt_r[:, :, osl], y01[:, :, lo:hi])

        def body():
            pending = None
            for sp in range(SP):
                xs = xts[sp]
                W = SP_W[sp]
                feats = {}

                # feature tiles are allocated at the max width (1024) so a
                # pool tag always recycles equal-size buffers; narrow
                # superpasses just use the first W columns.

                # -- range reduction + ACT passes for odd g --
                def reduce_g(g, j):
                    a, phat, c, b_s, b_c = _g_consts(g)
                    v = vpool.tile([INDIM, 1024], I32, tag="v")
                    nc.gpsimd.tensor_scalar(v[:, 0:W], xs, float(a), float(c),
                                            mybir.AluOpType.mult,
                                            mybir.AluOpType.add)
                    r = rpool.tile([INDIM, 1024], FP32, tag="r")
                    nc.vector.scalar_tensor_tensor(r[:, 0:W], v[:, 0:W],
                                                   float(-phat), xs,
                                                   mybir.AluOpType.mult,
                                                   mybir.AluOpType.add)
                    if g in DR_G:
                        # packed fp8 (sin, cos) pair for a DoubleRow block
                        f8 = f8pool.tile([INDIM, 2, 1024], FP8,
                                         tag=f"f8_{g}", name=f"f8_{g}")
                        nc.scalar.activation(f8[:, 0, 0:W], r[:, 0:W],
                                             mybir.ActivationFunctionType.Sin,
                                             bias=bt[:, 2 * j:2 * j + 1],
                                             scale=float(g))
                        nc.scalar.activation(f8[:, 1, 0:W], r[:, 0:W],
                                             mybir.ActivationFunctionType.Sin,
                                             bias=bt[:, 2 * j + 1:2 * j + 2],
                                             scale=float(g))
                        feats[("dr", g)] = f8
                        return
                    s = apool.tile([INDIM, 1024], FP16, tag="act")
                    nc.scalar.activation(s[:, 0:W], r[:, 0:W],
                                         mybir.ActivationFunctionType.Sin,
                                         bias=bt[:, 2 * j:2 * j + 1],
                                         scale=float(g))
                    cc = apool.tile([INDIM, 1024], FP16, tag="act")
                    nc.scalar.activation(cc[:, 0:W], r[:, 0:W],
                                         mybir.ActivationFunctionType.Sin,
                                         bias=bt[:, 2 * j + 1:2 * j + 2],
                                         scale=float(g))
                    feats[("s", g)] = s
                    feats[("c", g)] = cc

                def prod(name, m, in0, in1):
                    t = dpool.tile([INDIM, 1024], FP16, tag=f"{name}{m}")
                    nc.vector.tensor_tensor(t[:, 0:W], in0[:, 0:W],
                                            in1[:, 0:W],
                                            mybir.AluOpType.mult)
                    feats[(name, m)] = t
                    return t

                def truecos(m, vm, lam):
                    t = dpool.tile([INDIM, 1024], FP16, tag=f"C{2*m}")
                    nc.vector.tensor_scalar(t[:, 0:W], vm[:, 0:W],
                                            float(-lam), 1.0,
                                            mybir.AluOpType.mult,
                                            mybir.AluOpType.add)
                    return t

                # emission order = per-engine program order; keep DVE stream
                # interleaved so fmas feed ACT early and cascade flows.
                reduce_g(1, 0)
                reduce_g(3, 1)
                u1 = prod("u", 1, feats[("s", 1)], feats[("c", 1)])
                v1 = prod("v", 1, feats[("s", 1)], feats[("s", 1)])
                C2 = truecos(1, v1, LAMBDA[1])
                reduce_g(5, 2)
                u3 = prod("u", 3, feats[("s", 3)], feats[("c", 3)])
                v3 = prod("v", 3, feats[("s", 3)], feats[("s", 3)])
                C6 = truecos(3, v3, LAMBDA[3])
                reduce_g(7, 3)
                u5 = prod("u", 5, feats[("s", 5)], feats[("c", 5)])
                v5 = prod("v", 5, feats[("s", 5)], feats[("s", 5)])
                reduce_g(9, 4)
                u7 = prod("u", 7, feats[("s", 7)], feats[("c", 7)])
                v7 = prod("v", 7, feats[("s", 7)], feats[("s", 7)])
                reduce_g(11, 5)
                u2 = prod("u", 2, u1, C2)
                v2 = prod("v", 2, u1, u1)
                C4 = truecos(2, v2, LAMBDA[2])
                reduce_g(13, 6)
                u4 = prod("u", 4, u2, C4)
                v4 = prod("v", 4, u2, u2)
                C8 = truecos(4, v4, LAMBDA[4])
                reduce_g(15, 7)
                u6 = prod("u", 6, u3, C6)
                v6 = prod("v", 6, u3, u3)
                u8 = prod("u", 8, u4, C8)
                v8 = prod("v", 8, u4, u4)

                # previous superpass's eviction goes AFTER this superpass's
                # feature emission so ACT's program order is
                # [sp passes][sp+1 passes][evict sp] — eviction fires the
                # moment the PE frees the PSUM, without blocking features.
                if pending is not None:
                    emit_evict(*pending)

                # -- matmuls --
                n_kt = len(KT_ORDER)
                psum0 = ppool.tile([128, 1024], FP32, tag="p0")
                psum1 = ppool.tile([128, 1024], FP32, tag="p1")
                psums = [psum0, psum1]
                last = sp == SP - 1
                if last and pending is not None:
                    emit_evict(*pending)
                    pending = None
                # last superpass runs its two 512-column PSUM chunks as
                # sequential accumulation groups (chi-outer) so chunk 0's
                # eviction + store overlap chunk 1's matmuls -> short tail.
                # Other superpasses interleave chunks inside the kt loop.
                chi_outer = (0, 1) if last else (None,)
                for cho in chi_outer:
                    for kt, fkey in enumerate(KT_ORDER):
                        f = feats[fkey]
                        st, stp = (kt == 0), (kt == n_kt - 1)
                        chis = (cho,) if cho is not None else \
                            tuple(range(W // CH))
                        for oh in range(2):
                            if fkey[0] == "dr":
                                b = DR_G.index(fkey[1])
                                lhsT8 = w8t[:, :, (2 * b + oh) * 128:
                                            (2 * b + oh + 1) * 128]
                                for chi in chis:
                                    nc.tensor.matmul(
                                        psums[oh][:, chi * CH:(chi + 1) * CH],
                                        lhsT8, f[:, :, chi * CH:(chi + 1) * CH],
                                        start=st, stop=stp,
                                        perf_mode=mybir.MatmulPerfMode.DoubleRow,
                                    )
                                continue
                            kb = F16_IDX[fkey]
                            lhsT = wt[:, kb * OUTDIM + oh * 128:
                                      kb * OUTDIM + oh * 128 + 128]
                            for chi in chis:
                                nc.tensor.matmul(
                                    psums[oh][:, chi * CH:(chi + 1) * CH],
                                    lhsT, f[:, chi * CH:(chi + 1) * CH],
                                    start=st, stop=stp,
                                )
                    if cho is not None:
                        emit_evict(psums, sp, lo=cho * CH, hi=(cho + 1) * CH)
                if not last:
                    pending = (psums, sp)

        if reps == 1:
            body()
        else:
            with tc.For_i(0, reps, 1):
                body()

    nc.compile()
    _CACHED[key] = nc
    return nc


def _prep_inputs(x: np.ndarray, fouriercoeffs: np.ndarray, bias: np.ndarray):
    xt = np.ascontiguousarray(x.astype(np.float32, copy=False).T)  # (128, 32768)
    C0 = fouriercoeffs[0].astype(np.float32)   # (256, 128, 16) cos coeffs
    C1 = fouriercoeffs[1].astype(np.float32)   # sin coeffs

    # folded fp16 weight blocks: w_sb[i, kb*256 + col], col = output o
    w_sb = np.empty((INDIM, N_F16 * OUTDIM), np.float32)
    for (name, m), kb in F16_IDX.items():
        if name == "s":
            blk = C1[:, :, m - 1]                      # (o, i)
        elif name == "c":
            blk = C0[:, :, m - 1]
        elif name == "u":
            blk = KAPPA[m] * C1[:, :, 2 * m - 1]
        else:  # "v"
            blk = -LAMBDA[m] * C0[:, :, 2 * m - 1]
        w_sb[:, kb * OUTDIM:(kb + 1) * OUTDIM] = blk.T
    w_sb = w_sb.astype(np.float16)

    # fp8 DoubleRow weight pairs for DR_G: w8[i, j, (2b+oh)*128 + o'] with
    # j=0 matching the packed sin tile, j=1 the cos tile (unscaled e4m3:
    # subnormal step there ~= the normal-range step at |w|~0.02)
    import ml_dtypes
    w8 = np.empty((INDIM, 2, 2 * OUTDIM), np.float32)
    for b, g in enumerate(DR_G):
        for oh in range(2):
            cols = slice((2 * b + oh) * 128, (2 * b + oh + 1) * 128)
            w8[:, 0, cols] = C1[oh * 128:(oh + 1) * 128, :, g - 1].T
            w8[:, 1, cols] = C0[oh * 128:(oh + 1) * 128, :, g - 1].T
    w8 = w8.astype(ml_dtypes.float8_e4m3)

    # ACT bias table: 16 passes in order (b_s, b_c) per odd g
    bvals = np.empty(16, np.float32)
    for j, g in enumerate(ODD):
        _, _, _, b_s, b_c = _g_consts(g)
        bvals[2 * j] = b_s
        bvals[2 * j + 1] = b_c
    bt = np.tile(bvals[None, :], (INDIM, 1)).astype(np.float32)

    # folded output bias: bias + sum_i C0[o,i,2m-1] over even harmonics
    bias_fold = bias.reshape(-1).astype(np.float64).copy()
    for m in (1, 2, 3, 4, 5, 6, 7, 8):
        bias_fold += C0[:, :, 2 * m - 1].astype(np.float64).sum(axis=1)
    bias_sb = np.ascontiguousarray(
        bias_fold.astype(np.float32).reshape(2, 128).T)      # (128, 2)
    btb = np.ascontiguousarray(
        np.concatenate([bt, bias_sb], axis=1))               # (128, 18)
    return xt, w_sb, w8, btb


def kernel(x: np.ndarray, fouriercoeffs: np.ndarray, bias: np.ndarray,
           _trace: bool = False):
    x = np.asarray(x)
    fouriercoeffs = np.asarray(fouriercoeffs)
    bias = np.asarray(bias)
    orig_shape = x.shape
    x2 = x.reshape(-1, INDIM)
    assert x2.shape == (N_TOTAL, INDIM), x2.shape

    nc = _build()
    xt, w_sb, w8, btb = _prep_inputs(x2, fouriercoeffs, bias)
    in_maps = []
    for c in range(N_CORES):
        in_maps.append({
            "xt": np.ascontiguousarray(xt[:, c * N_SHARD:(c + 1) * N_SHARD]),
            "w": w_sb,
            "w8": w8,
            "btb": btb,
        })
    res = run_bass_kernel_spmd(nc, in_maps, list(range(N_CORES)),
                               trace=_trace)
    yt = np.concatenate([res.results[c]["yt"] for c in range(N_CORES)], axis=1)
    y = np.ascontiguousarray(yt.T).astype(np.float32)
    if _trace:
        kernel._last_result = res
    return y.reshape(*orig_shape[:-1], OUTDIM)



# revision 39
# speedup vs baseline: 1.0789x; 1.0166x over previous
"""KAN Fourier-linear kernel for 8 Trainium2 NeuronCores.

y[n,o] = sum_{i,g} C0[o,i,g]*cos(g*x[n,i]) + C1[o,i,g]*sin(g*x[n,i]) + bias[o]

Strategy (data-parallel over n, 4096 rows/core), double-angle cascade:
  - ACT (ScalarE) computes sin/cos only for odd g in {1,3,...,15} (16 Sin
    passes per superpass instead of 64), via the int-round range reduction:
      v   = int32(x*g/2pi + magic)        # gpsimd tensor_scalar
      r_g = x - v*(2pi/g)                 # DVE scalar_tensor_tensor (fp32)
      s_g = Sin(scale=g, bias=b_s)(r_g); c_g = Sin(scale=g, bias=b_c)(r_g)
  - Even harmonics come from 1-op DVE fp16 products with the scale factors
    folded into the weights host-side:
      u_m = s_m*c_m   covers sin(2m x) = kappa_m * u_m
      v_m = s_m*s_m   covers cos(2m x) = 1 - lambda_m * v_m
    (constants fold into the output bias). True-cos intermediates C_{2m} =
    1 - lambda*v_m (one tensor_scalar) extend the cascade to m in {2,4,6,8}.
  - Everything in fp16 (not bf16): the x128 weight folds amplify feature
    rounding error; fp16's 10-bit mantissa keeps rel err ~2e-3.
  - y.T tile = W.T @ F via PE, K=4096 accumulated in PSUM (fp16 inputs).
  - PSUM evicted by ACT Copy with fused per-partition bias add, fp16 out.
"""
import math
import numpy as np
from contextlib import ExitStack

import concourse.bass as bass
import concourse.mybir as mybir
import concourse.tile as tile
from concourse import bacc
from concourse.bass_utils import run_bass_kernel_spmd

N_CORES = 8
N_TOTAL = 32768
N_SHARD = N_TOTAL // N_CORES        # 4096 rows per core
INDIM = 128
OUTDIM = 256
GRID = 16
K_TOT = 2 * GRID * INDIM            # 4096
# superpass column widths: two narrow passes up front so the first
# feature chain (and hence the first matmul) starts early, wide passes
# for the bulk to keep per-instruction overhead low.
SP_W = (512, 512, 1024, 1024, 1024)
SP = len(SP_W)
SP_OFF = tuple(int(np.sum(SP_W[:i])) for i in range(SP + 1))
CH = 512                            # matmul moving chunk (PSUM bank limit)
TWO_PI = 2.0 * math.pi

FP32 = mybir.dt.float32
FP16 = mybir.dt.float16
FP8 = mybir.dt.float8e4
I32 = mybir.dt.int32

ODD = (1, 3, 5, 7, 9, 11, 13, 15)
# Harmonics whose (sin, cos) pair is packed into one fp8 tile and
# contracted with a single DoubleRow matmul (2 K-tiles per stream pass,
# 2x MAC rate).  4 of 32 K-blocks in fp8 keeps the extra quantization
# error at ~1.4e-2 max-rel (tolerance 2e-2); measured DR throughput is
# ~291 ns per N=512 DR-MM == the plain fp16 rate at twice the MACs.
DR_G = (9, 11, 13)
# kt consumption order: feature name per contraction block ("dr", g)
# entries consume a packed fp8 pair via one DoubleRow matmul.
# Interleaved so production (ACT for s/c, DVE for u/v) stays ahead of the PE.
KT_ORDER = [
    ("s", 1), ("c", 1), ("u", 1), ("v", 1),
    ("s", 3), ("c", 3), ("u", 3), ("v", 3),
    ("s", 5), ("c", 5), ("u", 5), ("v", 5),
    ("s", 7), ("c", 7), ("u", 7), ("v", 7),
    ("dr", 9), ("u", 2), ("v", 2),
    ("dr", 11), ("u", 4), ("v", 4),
    ("dr", 13), ("u", 6), ("v", 6),
    ("s", 15), ("c", 15), ("u", 8), ("v", 8),
]
# fp16 weight-block index for each non-dr entry, in order
F16_IDX = {}
for _e in KT_ORDER:
    if _e[0] != "dr":
        F16_IDX[_e] = len(F16_IDX)
N_F16 = len(F16_IDX)                # 28 fp16 K-blocks
KAPPA = {1: 2.0, 2: 4.0, 3: 2.0, 4: 8.0, 5: 2.0, 6: 4.0, 7: 2.0, 8: 16.0}
LAMBDA = {1: 2.0, 2: 8.0, 3: 2.0, 4: 32.0, 5: 2.0, 6: 8.0, 7: 2.0, 8: 128.0}


def _g_consts(g: int):
    a = np.float32(g / TWO_PI)
    phat = np.float32(TWO_PI / g)
    m = 2.0 ** math.ceil(math.log2(0.960 * g + 0.14))
    c = np.float32(m + 0.125)
    b_s = np.float32(m * g * float(phat))      # == 2pi*m up to fp32, matched to phat
    b_c = np.float32(float(b_s) + math.pi / 2.0)
    return a, phat, c, b_s, b_c


_CACHED = {}


def _build(reps: int = 1):
    key = ("nc", reps)
    if key in _CACHED:
        return _CACHED[key]
    nc = bacc.Bacc("TRN2", target_bir_lowering=False, debug=False,
                   num_devices=N_CORES)
    xt_d = nc.dram_tensor("xt", [INDIM, N_SHARD], FP32, kind="ExternalInput").ap()
    w_d = nc.dram_tensor("w", [INDIM, N_F16 * OUTDIM], FP16,
                         kind="ExternalInput").ap()
    w8_d = nc.dram_tensor("w8", [INDIM, 2, len(DR_G) * OUTDIM], FP8,
                          kind="ExternalInput").ap()
    btb_d = nc.dram_tensor("btb", [INDIM, 18], FP32, kind="ExternalInput").ap()
    yt_d = nc.dram_tensor("yt", [OUTDIM, N_SHARD], FP16, kind="ExternalOutput").ap()

    with tile.TileContext(nc) as tc, ExitStack() as ctx:
        cpool = ctx.enter_context(tc.tile_pool(name="const", bufs=1))
        vpool = ctx.enter_context(tc.tile_pool(name="v", bufs=6))
        rpool = ctx.enter_context(tc.tile_pool(name="r", bufs=4))
        apool = ctx.enter_context(tc.tile_pool(name="af", bufs=16))
        f8pool = ctx.enter_context(tc.tile_pool(name="f8", bufs=2))
        dpool = ctx.enter_context(tc.tile_pool(name="df", bufs=2))
        ypool = ctx.enter_context(tc.tile_pool(name="y", bufs=2))
        ppool = ctx.enter_context(tc.tile_pool(name="psum", bufs=2, space="PSUM"))

        # DMA priority order: the first matmul needs (a) the first weight
        # block and (b) features derived from x[sp0].  Land those first,
        # then interleave the rest so neither stream starves.
        xt = cpool.tile([INDIM, N_SHARD], FP32)
        wt = cpool.tile([INDIM, N_F16 * OUTDIM], FP16)
        w8t = cpool.tile([INDIM, 2, len(DR_G) * OUTDIM], FP8)
        btb = cpool.tile([INDIM, 18], FP32)
        bt = btb[:, 0:16]
        bias = btb[:, 16:18]
        # x chunk 0 first (it gates the whole feature chain), then the
        # ACT bias table, first weight block, and the rest interleaved.
        nc.sync.dma_start(xt[:, 0:512], xt_d[:, 0:512])
        nc.sync.dma_start(btb[:], btb_d[:])
        nc.sync.dma_start(wt[:, 0:2 * OUTDIM], w_d[:, 0:2 * OUTDIM])
        x_chunks = [(512, 1536), (1536, 2560), (2560, 3584), (3584, 4096)]
        w_slabs = [(2, 8), (8, 14), (14, 20), (20, 26)]
        for ci, ((xlo, xhi), (klo, khi)) in enumerate(zip(x_chunks, w_slabs)):
            nc.sync.dma_start(xt[:, xlo:xhi], xt_d[:, xlo:xhi])
            nc.sync.dma_start(wt[:, klo * OUTDIM:khi * OUTDIM],
                              w_d[:, klo * OUTDIM:khi * OUTDIM])
            if ci == 0:
                nc.sync.dma_start(w8t[:], w8_d[:])
        xts = [xt[:, SP_OFF[sp]:SP_OFF[sp + 1]] for sp in range(SP)]

        # view of yt for single-descriptor stores covering both row halves:
        # [r, h, c] -> yt[h*128 + r, c]
        yt_r = yt_d.rearrange("(h r) n -> r h n", h=2)

        def emit_evict(psums, sp, lo=0, hi=None):
            # split evictions across DVE (oh0) and ACT (oh1) so neither
            # queue eats both PSUM reads; store both halves with ONE DMA
            # descriptor (HWDGE descriptor processing is ~625ns, serial).
            # [lo,hi) selects a column chunk of the superpass.
            if hi is None:
                hi = SP_W[sp]
            osl = slice(SP_OFF[sp] + lo, SP_OFF[sp] + hi)
            y01 = ypool.tile([128, 2, 1024], FP16, tag="y01")
            nc.vector.tensor_scalar(y01[:, 0, lo:hi], psums[0][:, lo:hi],
                                    bias[:, 0:1], None,
                                    mybir.AluOpType.add)
            nc.scalar.activation(y01[:, 1, lo:hi], psums[1][:, lo:hi],
                                 mybir.ActivationFunctionType.Identity,
                                 bias=bias[:, 1:2], scale=1.0)
            nc.sync.dma_start(yt_r[:, :, osl], y01[:, :, lo:hi])

        def body():
            pending = None
            for sp in range(SP):
                xs = xts[sp]
                W = SP_W[sp]
                feats = {}

                # feature tiles are allocated at the max width (1024) so a
                # pool tag always recycles equal-size buffers; narrow
                # superpasses just use the first W columns.

                # -- range reduction + ACT passes for odd g --
                def reduce_g(g, j):
                    a, phat, c, b_s, b_c = _g_consts(g)
                    v = vpool.tile([INDIM, 1024], I32, tag="v")
                    nc.gpsimd.tensor_scalar(v[:, 0:W], xs, float(a), float(c),
                                            mybir.AluOpType.mult,
                                            mybir.AluOpType.add)
                    r = rpool.tile([INDIM, 1024], FP32, tag="r")
                    nc.vector.scalar_tensor_tensor(r[:, 0:W], v[:, 0:W],
                                                   float(-phat), xs,
                                                   mybir.AluOpType.mult,
                                                   mybir.AluOpType.add)
                    if g in DR_G:
                        # packed fp8 (sin, cos) pair for a DoubleRow block
                        f8 = f8pool.tile([INDIM, 2, 1024], FP8,
                                         tag=f"f8_{g}", name=f"f8_{g}")
                        nc.scalar.activation(f8[:, 0, 0:W], r[:, 0:W],
                                             mybir.ActivationFunctionType.Sin,
                                             bias=bt[:, 2 * j:2 * j + 1],
                                             scale=float(g))
                        nc.scalar.activation(f8[:, 1, 0:W], r[:, 0:W],
                                             mybir.ActivationFunctionType.Sin,
                                             bias=bt[:, 2 * j + 1:2 * j + 2],
                                             scale=float(g))
                        feats[("dr", g)] = f8
                        return
                    s = apool.tile([INDIM, 1024], FP16, tag="act")
                    nc.scalar.activation(s[:, 0:W], r[:, 0:W],
                                         mybir.ActivationFunctionType.Sin,
                                         bias=bt[:, 2 * j:2 * j + 1],
                                         scale=float(g))
                    cc = apool.tile([INDIM, 1024], FP16, tag="act")
                    nc.scalar.activation(cc[:, 0:W], r[:, 0:W],
                                         mybir.ActivationFunctionType.Sin,
                                         bias=bt[:, 2 * j + 1:2 * j + 2],
                                         scale=float(g))
                    feats[("s", g)] = s
                    feats[("c", g)] = cc

                def prod(name, m, in0, in1):
                    t = dpool.tile([INDIM, 1024], FP16, tag=f"{name}{m}")
                    nc.vector.tensor_tensor(t[:, 0:W], in0[:, 0:W],
                                            in1[:, 0:W],
                                            mybir.AluOpType.mult)
                    feats[(name, m)] = t
                    return t

                def truecos(m, vm, lam):
                    t = dpool.tile([INDIM, 1024], FP16, tag=f"C{2*m}")
                    nc.vector.tensor_scalar(t[:, 0:W], vm[:, 0:W],
                                            float(-lam), 1.0,
                                            mybir.AluOpType.mult,
                                            mybir.AluOpType.add)
                    return t

                # emission order = per-engine program order; keep DVE stream
                # interleaved so fmas feed ACT early and cascade flows.
                reduce_g(1, 0)
                reduce_g(3, 1)
                u1 = prod("u", 1, feats[("s", 1)], feats[("c", 1)])
                v1 = prod("v", 1, feats[("s", 1)], feats[("s", 1)])
                C2 = truecos(1, v1, LAMBDA[1])
                reduce_g(5, 2)
                u3 = prod("u", 3, feats[("s", 3)], feats[("c", 3)])
                v3 = prod("v", 3, feats[("s", 3)], feats[("s", 3)])
                C6 = truecos(3, v3, LAMBDA[3])
                reduce_g(7, 3)
                u5 = prod("u", 5, feats[("s", 5)], feats[("c", 5)])
                v5 = prod("v", 5, feats[("s", 5)], feats[("s", 5)])
                reduce_g(9, 4)
                u7 = prod("u", 7, feats[("s", 7)], feats[("c", 7)])
                v7 = prod("v", 7, feats[("s", 7)], feats[("s", 7)])
                reduce_g(11, 5)
                u2 = prod("u", 2, u1, C2)
                v2 = prod("v", 2, u1, u1)
                C4 = truecos(2, v2, LAMBDA[2])
                reduce_g(13, 6)
                u4 = prod("u", 4, u2, C4)
                v4 = prod("v", 4, u2, u2)
                C8 = truecos(4, v4, LAMBDA[4])
                reduce_g(15, 7)
                u6 = prod("u", 6, u3, C6)
                v6 = prod("v", 6, u3, u3)
                u8 = prod("u", 8, u4, C8)
                v8 = prod("v", 8, u4, u4)

                # previous superpass's eviction goes AFTER this superpass's
                # feature emission so ACT's program order is
                # [sp passes][sp+1 passes][evict sp] — eviction fires the
                # moment the PE frees the PSUM, without blocking features.
                if pending is not None:
                    emit_evict(*pending)

                # -- matmuls --
                n_kt = len(KT_ORDER)
                psum0 = ppool.tile([128, 1024], FP32, tag="p0")
                psum1 = ppool.tile([128, 1024], FP32, tag="p1")
                psums = [psum0, psum1]
                last = sp == SP - 1
                if last and pending is not None:
                    emit_evict(*pending)
                    pending = None
                # last superpass runs its two 512-column PSUM chunks as
                # sequential accumulation groups (chi-outer) so chunk 0's
                # eviction + store overlap chunk 1's matmuls -> short tail.
                # Other superpasses interleave chunks inside the kt loop.
                chi_outer = (0, 1) if last else (None,)
                for cho in chi_outer:
                    for kt, fkey in enumerate(KT_ORDER):
                        f = feats[fkey]
                        st, stp = (kt == 0), (kt == n_kt - 1)
                        chis = (cho,) if cho is not None else \
                            tuple(range(W // CH))
                        for oh in range(2):
                            if fkey[0] == "dr":
                                b = DR_G.index(fkey[1])
                                lhsT8 = w8t[:, :, (2 * b + oh) * 128:
                                            (2 * b + oh + 1) * 128]
                                for chi in chis:
                                    nc.tensor.matmul(
                                        psums[oh][:, chi * CH:(chi + 1) * CH],
                                        lhsT8, f[:, :, chi * CH:(chi + 1) * CH],
                                        start=st, stop=stp,
                                        perf_mode=mybir.MatmulPerfMode.DoubleRow,
                                    )
                                continue
                            kb = F16_IDX[fkey]
                            lhsT = wt[:, kb * OUTDIM + oh * 128:
                                      kb * OUTDIM + oh * 128 + 128]
                            for chi in chis:
                                nc.tensor.matmul(
                                    psums[oh][:, chi * CH:(chi + 1) * CH],
                                    lhsT, f[:, chi * CH:(chi + 1) * CH],
                                    start=st, stop=stp,
                                )
                    if cho is not None:
                        emit_evict(psums, sp, lo=cho * CH, hi=(cho + 1) * CH)
                if not last:
                    pending = (psums, sp)

        if reps == 1:
            body()
        else:
            with tc.For_i(0, reps, 1):
                body()

    nc.compile()
    _CACHED[key] = nc
    return nc


def _prep_inputs(x: np.ndarray, fouriercoeffs: np.ndarray, bias: np.ndarray):
    xt = np.ascontiguousarray(x.astype(np.float32, copy=False).T)  # (128, 32768)
    C0 = fouriercoeffs[0].astype(np.float32)   # (256, 128, 16) cos coeffs
    C1 = fouriercoeffs[1].astype(np.float32)   # sin coeffs

    # folded fp16 weight blocks: w_sb[i, kb*256 + col], col = output o
    w_sb = np.empty((INDIM, N_F16 * OUTDIM), np.float32)
    for (name, m), kb in F16_IDX.items():
        if name == "s":
            blk = C1[:, :, m - 1]                      # (o, i)
        elif name == "c":
            blk = C0[:, :, m - 1]
        elif name == "u":
            blk = KAPPA[m] * C1[:, :, 2 * m - 1]
        else:  # "v"
            blk = -LAMBDA[m] * C0[:, :, 2 * m - 1]
        w_sb[:, kb * OUTDIM:(kb + 1) * OUTDIM] = blk.T
    w_sb = w_sb.astype(np.float16)

    # fp8 DoubleRow weight pairs for DR_G: w8[i, j, (2b+oh)*128 + o'] with
    # j=0 matching the packed sin tile, j=1 the cos tile (unscaled e4m3:
    # subnormal step there ~= the normal-range step at |w|~0.02)
    import ml_dtypes
    w8 = np.empty((INDIM, 2, len(DR_G) * OUTDIM), np.float32)
    for b, g in enumerate(DR_G):
        for oh in range(2):
            cols = slice((2 * b + oh) * 128, (2 * b + oh + 1) * 128)
            w8[:, 0, cols] = C1[oh * 128:(oh + 1) * 128, :, g - 1].T
            w8[:, 1, cols] = C0[oh * 128:(oh + 1) * 128, :, g - 1].T
    w8 = w8.astype(ml_dtypes.float8_e4m3)

    # ACT bias table: 16 passes in order (b_s, b_c) per odd g
    bvals = np.empty(16, np.float32)
    for j, g in enumerate(ODD):
        _, _, _, b_s, b_c = _g_consts(g)
        bvals[2 * j] = b_s
        bvals[2 * j + 1] = b_c
    bt = np.tile(bvals[None, :], (INDIM, 1)).astype(np.float32)

    # folded output bias: bias + sum_i C0[o,i,2m-1] over even harmonics
    bias_fold = bias.reshape(-1).astype(np.float64).copy()
    for m in (1, 2, 3, 4, 5, 6, 7, 8):
        bias_fold += C0[:, :, 2 * m - 1].astype(np.float64).sum(axis=1)
    bias_sb = np.ascontiguousarray(
        bias_fold.astype(np.float32).reshape(2, 128).T)      # (128, 2)
    btb = np.ascontiguousarray(
        np.concatenate([bt, bias_sb], axis=1))               # (128, 18)
    return xt, w_sb, w8, btb


def kernel(x: np.ndarray, fouriercoeffs: np.ndarray, bias: np.ndarray,
           _trace: bool = False):
    x = np.asarray(x)
    fouriercoeffs = np.asarray(fouriercoeffs)
    bias = np.asarray(bias)
    orig_shape = x.shape
    x2 = x.reshape(-1, INDIM)
    assert x2.shape == (N_TOTAL, INDIM), x2.shape

    nc = _build()
    xt, w_sb, w8, btb = _prep_inputs(x2, fouriercoeffs, bias)
    in_maps = []
    for c in range(N_CORES):
        in_maps.append({
            "xt": np.ascontiguousarray(xt[:, c * N_SHARD:(c + 1) * N_SHARD]),
            "w": w_sb,
            "w8": w8,
            "btb": btb,
        })
    res = run_bass_kernel_spmd(nc, in_maps, list(range(N_CORES)),
                               trace=_trace)
    yt = np.concatenate([res.results[c]["yt"] for c in range(N_CORES)], axis=1)
    y = np.ascontiguousarray(yt.T).astype(np.float32)
    if _trace:
        kernel._last_result = res
    return y.reshape(*orig_shape[:-1], OUTDIM)



# revision 40
# speedup vs baseline: 1.0798x; 1.0008x over previous
"""KAN Fourier-linear kernel for 8 Trainium2 NeuronCores.

y[n,o] = sum_{i,g} C0[o,i,g]*cos(g*x[n,i]) + C1[o,i,g]*sin(g*x[n,i]) + bias[o]

Strategy (data-parallel over n, 4096 rows/core), double-angle cascade:
  - ACT (ScalarE) computes sin/cos only for odd g in {1,3,...,15} (16 Sin
    passes per superpass instead of 64), via the int-round range reduction:
      v   = int32(x*g/2pi + magic)        # gpsimd tensor_scalar
      r_g = x - v*(2pi/g)                 # DVE scalar_tensor_tensor (fp32)
      s_g = Sin(scale=g, bias=b_s)(r_g); c_g = Sin(scale=g, bias=b_c)(r_g)
  - Even harmonics come from 1-op DVE fp16 products with the scale factors
    folded into the weights host-side:
      u_m = s_m*c_m   covers sin(2m x) = kappa_m * u_m
      v_m = s_m*s_m   covers cos(2m x) = 1 - lambda_m * v_m
    (constants fold into the output bias). True-cos intermediates C_{2m} =
    1 - lambda*v_m (one tensor_scalar) extend the cascade to m in {2,4,6,8}.
  - Everything in fp16 (not bf16): the x128 weight folds amplify feature
    rounding error; fp16's 10-bit mantissa keeps rel err ~2e-3.
  - y.T tile = W.T @ F via PE, K=4096 accumulated in PSUM (fp16 inputs).
  - PSUM evicted by ACT Copy with fused per-partition bias add, fp16 out.
"""
import math
import numpy as np
from contextlib import ExitStack

import concourse.bass as bass
import concourse.mybir as mybir
import concourse.tile as tile
from concourse import bacc
from concourse.bass_utils import run_bass_kernel_spmd

N_CORES = 8
N_TOTAL = 32768
N_SHARD = N_TOTAL // N_CORES        # 4096 rows per core
INDIM = 128
OUTDIM = 256
GRID = 16
K_TOT = 2 * GRID * INDIM            # 4096
# superpass column widths: two narrow passes up front so the first
# feature chain (and hence the first matmul) starts early, wide passes
# for the bulk to keep per-instruction overhead low.
SP_W = (512, 512, 1024, 1024, 1024)
SP = len(SP_W)
SP_OFF = tuple(int(np.sum(SP_W[:i])) for i in range(SP + 1))
CH = 512                            # matmul moving chunk (PSUM bank limit)
TWO_PI = 2.0 * math.pi

FP32 = mybir.dt.float32
FP16 = mybir.dt.float16
FP8 = mybir.dt.float8e4
I32 = mybir.dt.int32

ODD = (1, 3, 5, 7, 9, 11, 13, 15)
# Harmonics whose (sin, cos) pair is packed into one fp8 tile and
# contracted with a single DoubleRow matmul (2 K-tiles per stream pass,
# 2x MAC rate).  4 of 32 K-blocks in fp8 keeps the extra quantization
# error at ~1.4e-2 max-rel (tolerance 2e-2); measured DR throughput is
# ~291 ns per N=512 DR-MM == the plain fp16 rate at twice the MACs.
DR_G = (9, 11, 13)
# kt consumption order: feature name per contraction block ("dr", g)
# entries consume a packed fp8 pair via one DoubleRow matmul.
# Interleaved so production (ACT for s/c, DVE for u/v) stays ahead of the PE.
KT_ORDER = [
    ("s", 1), ("c", 1), ("u", 1), ("v", 1),
    ("s", 3), ("c", 3), ("u", 3), ("v", 3),
    ("s", 5), ("c", 5), ("u", 5), ("v", 5),
    ("s", 7), ("c", 7), ("u", 7), ("v", 7),
    ("dr", 9), ("u", 2), ("v", 2),
    ("dr", 11), ("u", 4), ("v", 4),
    ("dr", 13), ("u", 6), ("v", 6),
    ("s", 15), ("c", 15), ("u", 8), ("v", 8),
]
# fp16 weight-block index for each non-dr entry, in order
F16_IDX = {}
for _e in KT_ORDER:
    if _e[0] != "dr":
        F16_IDX[_e] = len(F16_IDX)
N_F16 = len(F16_IDX)                # 28 fp16 K-blocks
KAPPA = {1: 2.0, 2: 4.0, 3: 2.0, 4: 8.0, 5: 2.0, 6: 4.0, 7: 2.0, 8: 16.0}
LAMBDA = {1: 2.0, 2: 8.0, 3: 2.0, 4: 32.0, 5: 2.0, 6: 8.0, 7: 2.0, 8: 128.0}


def _g_consts(g: int):
    a = np.float32(g / TWO_PI)
    phat = np.float32(TWO_PI / g)
    m = 2.0 ** math.ceil(math.log2(0.960 * g + 0.14))
    c = np.float32(m + 0.125)
    b_s = np.float32(m * g * float(phat))      # == 2pi*m up to fp32, matched to phat
    b_c = np.float32(float(b_s) + math.pi / 2.0)
    return a, phat, c, b_s, b_c


_CACHED = {}


def _build(reps: int = 1):
    key = ("nc", reps)
    if key in _CACHED:
        return _CACHED[key]
    nc = bacc.Bacc("TRN2", target_bir_lowering=False, debug=False,
                   num_devices=N_CORES)
    xt_d = nc.dram_tensor("xt", [INDIM, N_SHARD], FP32, kind="ExternalInput").ap()
    w_d = nc.dram_tensor("w", [INDIM, N_F16 * OUTDIM], FP16,
                         kind="ExternalInput").ap()
    w8_d = nc.dram_tensor("w8", [INDIM, 2, len(DR_G) * OUTDIM], FP8,
                          kind="ExternalInput").ap()
    btb_d = nc.dram_tensor("btb", [INDIM, 18], FP32, kind="ExternalInput").ap()
    yt_d = nc.dram_tensor("yt", [OUTDIM, N_SHARD], FP16, kind="ExternalOutput").ap()

    with tile.TileContext(nc) as tc, ExitStack() as ctx:
        cpool = ctx.enter_context(tc.tile_pool(name="const", bufs=1))
        vpool = ctx.enter_context(tc.tile_pool(name="v", bufs=6))
        rpool = ctx.enter_context(tc.tile_pool(name="r", bufs=4))
        apool = ctx.enter_context(tc.tile_pool(name="af", bufs=16))
        f8pool = ctx.enter_context(tc.tile_pool(name="f8", bufs=2))
        dpool = ctx.enter_context(tc.tile_pool(name="df", bufs=2))
        ypool = ctx.enter_context(tc.tile_pool(name="y", bufs=2))
        ppool = ctx.enter_context(tc.tile_pool(name="psum", bufs=2, space="PSUM"))

        # DMA priority order: the first matmul needs (a) the first weight
        # block and (b) features derived from x[sp0].  Land those first,
        # then interleave the rest so neither stream starves.
        xt = cpool.tile([INDIM, N_SHARD], FP32)
        wt = cpool.tile([INDIM, N_F16 * OUTDIM], FP16)
        w8t = cpool.tile([INDIM, 2, len(DR_G) * OUTDIM], FP8)
        btb = cpool.tile([INDIM, 18], FP32)
        bt = btb[:, 0:16]
        bias = btb[:, 16:18]
        # x chunk 0 first (it gates the whole feature chain), then the
        # ACT bias table, first weight block, and the rest interleaved.
        nc.sync.dma_start(xt[:, 0:512], xt_d[:, 0:512])
        nc.sync.dma_start(btb[:], btb_d[:])
        nc.sync.dma_start(wt[:, 0:2 * OUTDIM], w_d[:, 0:2 * OUTDIM])
        x_chunks = [(512, 1536), (1536, 2560), (2560, 3584), (3584, 4096)]
        w_slabs = [(2, 8), (8, 14), (14, 20), (20, 26)]
        for ci, ((xlo, xhi), (klo, khi)) in enumerate(zip(x_chunks, w_slabs)):
            nc.sync.dma_start(xt[:, xlo:xhi], xt_d[:, xlo:xhi])
            nc.sync.dma_start(wt[:, klo * OUTDIM:khi * OUTDIM],
                              w_d[:, klo * OUTDIM:khi * OUTDIM])
            if ci == 0:
                nc.sync.dma_start(w8t[:], w8_d[:])
        xts = [xt[:, SP_OFF[sp]:SP_OFF[sp + 1]] for sp in range(SP)]

        # view of yt for single-descriptor stores covering both row halves:
        # [r, h, c] -> yt[h*128 + r, c]
        yt_r = yt_d.rearrange("(h r) n -> r h n", h=2)

        def emit_evict(psums, sp, lo=0, hi=None):
            # split evictions across DVE (oh0) and ACT (oh1) so neither
            # queue eats both PSUM reads; store both halves with ONE DMA
            # descriptor (HWDGE descriptor processing is ~625ns, serial).
            # [lo,hi) selects a column chunk of the superpass.
            if hi is None:
                hi = SP_W[sp]
            osl = slice(SP_OFF[sp] + lo, SP_OFF[sp] + hi)
            y01 = ypool.tile([128, 2, 1024], FP16, tag="y01")
            nc.vector.tensor_scalar(y01[:, 0, lo:hi], psums[0][:, lo:hi],
                                    bias[:, 0:1], None,
                                    mybir.AluOpType.add)
            nc.scalar.activation(y01[:, 1, lo:hi], psums[1][:, lo:hi],
                                 mybir.ActivationFunctionType.Identity,
                                 bias=bias[:, 1:2], scale=1.0)
            nc.sync.dma_start(yt_r[:, :, osl], y01[:, :, lo:hi])

        def body():
            pending = None
            for sp in range(SP):
                xs = xts[sp]
                W = SP_W[sp]
                feats = {}

                # feature tiles are allocated at the max width (1024) so a
                # pool tag always recycles equal-size buffers; narrow
                # superpasses just use the first W columns.

                # -- range reduction + ACT passes for odd g --
                def reduce_g(g, j):
                    a, phat, c, b_s, b_c = _g_consts(g)
                    v = vpool.tile([INDIM, 1024], I32, tag="v")
                    nc.gpsimd.tensor_scalar(v[:, 0:W], xs, float(a), float(c),
                                            mybir.AluOpType.mult,
                                            mybir.AluOpType.add)
                    r = rpool.tile([INDIM, 1024], FP32, tag="r")
                    nc.vector.scalar_tensor_tensor(r[:, 0:W], v[:, 0:W],
                                                   float(-phat), xs,
                                                   mybir.AluOpType.mult,
                                                   mybir.AluOpType.add)
                    if g in DR_G:
                        # packed fp8 (sin, cos) pair for a DoubleRow block
                        f8 = f8pool.tile([INDIM, 2, 1024], FP8,
                                         tag=f"f8_{g}", name=f"f8_{g}")
                        nc.scalar.activation(f8[:, 0, 0:W], r[:, 0:W],
                                             mybir.ActivationFunctionType.Sin,
                                             bias=bt[:, 2 * j:2 * j + 1],
                                             scale=float(g))
                        nc.scalar.activation(f8[:, 1, 0:W], r[:, 0:W],
                                             mybir.ActivationFunctionType.Sin,
                                             bias=bt[:, 2 * j + 1:2 * j + 2],
                                             scale=float(g))
                        feats[("dr", g)] = f8
                        return
                    s = apool.tile([INDIM, 1024], FP16, tag="act")
                    nc.scalar.activation(s[:, 0:W], r[:, 0:W],
                                         mybir.ActivationFunctionType.Sin,
                                         bias=bt[:, 2 * j:2 * j + 1],
                                         scale=float(g))
                    cc = apool.tile([INDIM, 1024], FP16, tag="act")
                    nc.scalar.activation(cc[:, 0:W], r[:, 0:W],
                                         mybir.ActivationFunctionType.Sin,
                                         bias=bt[:, 2 * j + 1:2 * j + 2],
                                         scale=float(g))
                    feats[("s", g)] = s
                    feats[("c", g)] = cc

                def prod(name, m, in0, in1):
                    t = dpool.tile([INDIM, 1024], FP16, tag=f"{name}{m}")
                    nc.vector.tensor_tensor(t[:, 0:W], in0[:, 0:W],
                                            in1[:, 0:W],
                                            mybir.AluOpType.mult)
                    feats[(name, m)] = t
                    return t

                def truecos(m, vm, lam):
                    t = dpool.tile([INDIM, 1024], FP16, tag=f"C{2*m}")
                    nc.vector.tensor_scalar(t[:, 0:W], vm[:, 0:W],
                                            float(-lam), 1.0,
                                            mybir.AluOpType.mult,
                                            mybir.AluOpType.add)
                    return t

                # emission order = per-engine program order; keep DVE stream
                # interleaved so fmas feed ACT early and cascade flows.
                reduce_g(1, 0)
                reduce_g(3, 1)
                u1 = prod("u", 1, feats[("s", 1)], feats[("c", 1)])
                v1 = prod("v", 1, feats[("s", 1)], feats[("s", 1)])
                C2 = truecos(1, v1, LAMBDA[1])
                reduce_g(5, 2)
                u3 = prod("u", 3, feats[("s", 3)], feats[("c", 3)])
                v3 = prod("v", 3, feats[("s", 3)], feats[("s", 3)])
                C6 = truecos(3, v3, LAMBDA[3])
                reduce_g(7, 3)
                u5 = prod("u", 5, feats[("s", 5)], feats[("c", 5)])
                v5 = prod("v", 5, feats[("s", 5)], feats[("s", 5)])
                reduce_g(9, 4)
                u7 = prod("u", 7, feats[("s", 7)], feats[("c", 7)])
                v7 = prod("v", 7, feats[("s", 7)], feats[("s", 7)])
                reduce_g(11, 5)
                u2 = prod("u", 2, u1, C2)
                v2 = prod("v", 2, u1, u1)
                C4 = truecos(2, v2, LAMBDA[2])
                reduce_g(13, 6)
                u4 = prod("u", 4, u2, C4)
                v4 = prod("v", 4, u2, u2)
                C8 = truecos(4, v4, LAMBDA[4])
                reduce_g(15, 7)
                u6 = prod("u", 6, u3, C6)
                v6 = prod("v", 6, u3, u3)
                u8 = prod("u", 8, u4, C8)
                v8 = prod("v", 8, u4, u4)

                # previous superpass's eviction goes AFTER this superpass's
                # feature emission so ACT's program order is
                # [sp passes][sp+1 passes][evict sp] — eviction fires the
                # moment the PE frees the PSUM, without blocking features.
                if pending is not None:
                    emit_evict(*pending)

                # -- matmuls --
                n_kt = len(KT_ORDER)
                psum0 = ppool.tile([128, 1024], FP32, tag="p0")
                psum1 = ppool.tile([128, 1024], FP32, tag="p1")
                psums = [psum0, psum1]
                last = sp == SP - 1
                if last and pending is not None:
                    emit_evict(*pending)
                    pending = None
                for kt, fkey in enumerate(KT_ORDER):
                    f = feats[fkey]
                    st, stp = (kt == 0), (kt == n_kt - 1)
                    for oh in range(2):
                        if fkey[0] == "dr":
                            b = DR_G.index(fkey[1])
                            lhsT8 = w8t[:, :, (2 * b + oh) * 128:
                                        (2 * b + oh + 1) * 128]
                            for chi in range(W // CH):
                                nc.tensor.matmul(
                                    psums[oh][:, chi * CH:(chi + 1) * CH],
                                    lhsT8, f[:, :, chi * CH:(chi + 1) * CH],
                                    start=st, stop=stp,
                                    perf_mode=mybir.MatmulPerfMode.DoubleRow,
                                )
                            continue
                        kb = F16_IDX[fkey]
                        lhsT = wt[:, kb * OUTDIM + oh * 128:
                                  kb * OUTDIM + oh * 128 + 128]
                        for chi in range(W // CH):
                            nc.tensor.matmul(
                                psums[oh][:, chi * CH:(chi + 1) * CH],
                                lhsT, f[:, chi * CH:(chi + 1) * CH],
                                start=st, stop=stp,
                            )
                if not last:
                    pending = (psums, sp)
                else:
                    # evict per 512-column chunk so the first chunk's
                    # eviction + store start while the final matmuls of
                    # the second chunk are still draining.
                    for chi in range(W // CH):
                        emit_evict(psums, sp, lo=chi * CH, hi=(chi + 1) * CH)

        if reps == 1:
            body()
        else:
            with tc.For_i(0, reps, 1):
                body()

    nc.compile()
    _CACHED[key] = nc
    return nc


def _prep_inputs(x: np.ndarray, fouriercoeffs: np.ndarray, bias: np.ndarray):
    xt = np.ascontiguousarray(x.astype(np.float32, copy=False).T)  # (128, 32768)
    C0 = fouriercoeffs[0].astype(np.float32)   # (256, 128, 16) cos coeffs
    C1 = fouriercoeffs[1].astype(np.float32)   # sin coeffs

    # folded fp16 weight blocks: w_sb[i, kb*256 + col], col = output o
    w_sb = np.empty((INDIM, N_F16 * OUTDIM), np.float32)
    for (name, m), kb in F16_IDX.items():
        if name == "s":
            blk = C1[:, :, m - 1]                      # (o, i)
        elif name == "c":
            blk = C0[:, :, m - 1]
        elif name == "u":
            blk = KAPPA[m] * C1[:, :, 2 * m - 1]
        else:  # "v"
            blk = -LAMBDA[m] * C0[:, :, 2 * m - 1]
        w_sb[:, kb * OUTDIM:(kb + 1) * OUTDIM] = blk.T
    w_sb = w_sb.astype(np.float16)

    # fp8 DoubleRow weight pairs for DR_G: w8[i, j, (2b+oh)*128 + o'] with
    # j=0 matching the packed sin tile, j=1 the cos tile (unscaled e4m3:
    # subnormal step there ~= the normal-range step at |w|~0.02)
    import ml_dtypes
    w8 = np.empty((INDIM, 2, len(DR_G) * OUTDIM), np.float32)
    for b, g in enumerate(DR_G):
        for oh in range(2):
            cols = slice((2 * b + oh) * 128, (2 * b + oh + 1) * 128)
            w8[:, 0, cols] = C1[oh * 128:(oh + 1) * 128, :, g - 1].T
            w8[:, 1, cols] = C0[oh * 128:(oh + 1) * 128, :, g - 1].T
    w8 = w8.astype(ml_dtypes.float8_e4m3)

    # ACT bias table: 16 passes in order (b_s, b_c) per odd g
    bvals = np.empty(16, np.float32)
    for j, g in enumerate(ODD):
        _, _, _, b_s, b_c = _g_consts(g)
        bvals[2 * j] = b_s
        bvals[2 * j + 1] = b_c
    bt = np.tile(bvals[None, :], (INDIM, 1)).astype(np.float32)

    # folded output bias: bias + sum_i C0[o,i,2m-1] over even harmonics
    bias_fold = bias.reshape(-1).astype(np.float64).copy()
    for m in (1, 2, 3, 4, 5, 6, 7, 8):
        bias_fold += C0[:, :, 2 * m - 1].astype(np.float64).sum(axis=1)
    bias_sb = np.ascontiguousarray(
        bias_fold.astype(np.float32).reshape(2, 128).T)      # (128, 2)
    btb = np.ascontiguousarray(
        np.concatenate([bt, bias_sb], axis=1))               # (128, 18)
    return xt, w_sb, w8, btb


def kernel(x: np.ndarray, fouriercoeffs: np.ndarray, bias: np.ndarray,
           _trace: bool = False):
    x = np.asarray(x)
    fouriercoeffs = np.asarray(fouriercoeffs)
    bias = np.asarray(bias)
    orig_shape = x.shape
    x2 = x.reshape(-1, INDIM)
    assert x2.shape == (N_TOTAL, INDIM), x2.shape

    nc = _build()
    xt, w_sb, w8, btb = _prep_inputs(x2, fouriercoeffs, bias)
    in_maps = []
    for c in range(N_CORES):
        in_maps.append({
            "xt": np.ascontiguousarray(xt[:, c * N_SHARD:(c + 1) * N_SHARD]),
            "w": w_sb,
            "w8": w8,
            "btb": btb,
        })
    res = run_bass_kernel_spmd(nc, in_maps, list(range(N_CORES)),
                               trace=_trace)
    yt = np.concatenate([res.results[c]["yt"] for c in range(N_CORES)], axis=1)
    y = np.ascontiguousarray(yt.T).astype(np.float32)
    if _trace:
        kernel._last_result = res
    return y.reshape(*orig_shape[:-1], OUTDIM)



# revision 41
# speedup vs baseline: 1.0825x; 1.0025x over previous
"""KAN Fourier-linear kernel for 8 Trainium2 NeuronCores.

y[n,o] = sum_{i,g} C0[o,i,g]*cos(g*x[n,i]) + C1[o,i,g]*sin(g*x[n,i]) + bias[o]

Strategy (data-parallel over n, 4096 rows/core), double-angle cascade:
  - ACT (ScalarE) computes sin/cos only for odd g in {1,3,...,15} (16 Sin
    passes per superpass instead of 64), via the int-round range reduction:
      v   = int32(x*g/2pi + magic)        # gpsimd tensor_scalar
      r_g = x - v*(2pi/g)                 # DVE scalar_tensor_tensor (fp32)
      s_g = Sin(scale=g, bias=b_s)(r_g); c_g = Sin(scale=g, bias=b_c)(r_g)
  - Even harmonics come from 1-op DVE fp16 products with the scale factors
    folded into the weights host-side:
      u_m = s_m*c_m   covers sin(2m x) = kappa_m * u_m
      v_m = s_m*s_m   covers cos(2m x) = 1 - lambda_m * v_m
    (constants fold into the output bias). True-cos intermediates C_{2m} =
    1 - lambda*v_m (one tensor_scalar) extend the cascade to m in {2,4,6,8}.
  - Everything in fp16 (not bf16): the x128 weight folds amplify feature
    rounding error; fp16's 10-bit mantissa keeps rel err ~2e-3.
  - y.T tile = W.T @ F via PE, K=4096 accumulated in PSUM (fp16 inputs).
  - PSUM evicted by ACT Copy with fused per-partition bias add, fp16 out.
"""
import math
import numpy as np
from contextlib import ExitStack

import concourse.bass as bass
import concourse.mybir as mybir
import concourse.tile as tile
from concourse import bacc
from concourse.bass_utils import run_bass_kernel_spmd

N_CORES = 8
N_TOTAL = 32768
N_SHARD = N_TOTAL // N_CORES        # 4096 rows per core
INDIM = 128
OUTDIM = 256
GRID = 16
K_TOT = 2 * GRID * INDIM            # 4096
# superpass column widths: two narrow passes up front so the first
# feature chain (and hence the first matmul) starts early, wide passes
# for the bulk to keep per-instruction overhead low.
SP_W = (512, 512, 1024, 1024, 1024)
SP = len(SP_W)
SP_OFF = tuple(int(np.sum(SP_W[:i])) for i in range(SP + 1))
CH = 512                            # matmul moving chunk (PSUM bank limit)
TWO_PI = 2.0 * math.pi

FP32 = mybir.dt.float32
FP16 = mybir.dt.float16
FP8 = mybir.dt.float8e4
I32 = mybir.dt.int32

ODD = (1, 3, 5, 7, 9, 11, 13, 15)
# Harmonics whose (sin, cos) pair is packed into one fp8 tile and
# contracted with a single DoubleRow matmul (2 K-tiles per stream pass,
# 2x MAC rate).  4 of 32 K-blocks in fp8 keeps the extra quantization
# error at ~1.4e-2 max-rel (tolerance 2e-2); measured DR throughput is
# ~291 ns per N=512 DR-MM == the plain fp16 rate at twice the MACs.
DR_G = (9, 11, 13)
# kt consumption order: feature name per contraction block ("dr", g)
# entries consume a packed fp8 pair via one DoubleRow matmul.
# Interleaved so production (ACT for s/c, DVE for u/v) stays ahead of the PE.
KT_ORDER = [
    ("s", 1), ("c", 1), ("u", 1), ("v", 1),
    ("s", 3), ("c", 3), ("u", 3), ("v", 3),
    ("s", 5), ("c", 5), ("u", 5), ("v", 5),
    ("s", 7), ("c", 7), ("u", 7), ("v", 7),
    ("dr", 9), ("dr", 11), ("dr", 13),
    ("u", 2), ("v", 2), ("u", 4), ("v", 4), ("u", 6), ("v", 6),
    ("s", 15), ("c", 15), ("u", 8), ("v", 8),
]
# fp16 weight-block index for each non-dr entry, in order
F16_IDX = {}
for _e in KT_ORDER:
    if _e[0] != "dr":
        F16_IDX[_e] = len(F16_IDX)
N_F16 = len(F16_IDX)                # 28 fp16 K-blocks
KAPPA = {1: 2.0, 2: 4.0, 3: 2.0, 4: 8.0, 5: 2.0, 6: 4.0, 7: 2.0, 8: 16.0}
LAMBDA = {1: 2.0, 2: 8.0, 3: 2.0, 4: 32.0, 5: 2.0, 6: 8.0, 7: 2.0, 8: 128.0}


def _g_consts(g: int):
    a = np.float32(g / TWO_PI)
    phat = np.float32(TWO_PI / g)
    m = 2.0 ** math.ceil(math.log2(0.960 * g + 0.14))
    c = np.float32(m + 0.125)
    b_s = np.float32(m * g * float(phat))      # == 2pi*m up to fp32, matched to phat
    b_c = np.float32(float(b_s) + math.pi / 2.0)
    return a, phat, c, b_s, b_c


_CACHED = {}


def _build(reps: int = 1):
    key = ("nc", reps)
    if key in _CACHED:
        return _CACHED[key]
    nc = bacc.Bacc("TRN2", target_bir_lowering=False, debug=False,
                   num_devices=N_CORES)
    xt_d = nc.dram_tensor("xt", [INDIM, N_SHARD], FP32, kind="ExternalInput").ap()
    w_d = nc.dram_tensor("w", [INDIM, N_F16 * OUTDIM], FP16,
                         kind="ExternalInput").ap()
    w8_d = nc.dram_tensor("w8", [INDIM, 2, len(DR_G) * OUTDIM], FP8,
                          kind="ExternalInput").ap()
    btb_d = nc.dram_tensor("btb", [INDIM, 18], FP32, kind="ExternalInput").ap()
    yt_d = nc.dram_tensor("yt", [OUTDIM, N_SHARD], FP16, kind="ExternalOutput").ap()

    with tile.TileContext(nc) as tc, ExitStack() as ctx:
        cpool = ctx.enter_context(tc.tile_pool(name="const", bufs=1))
        vpool = ctx.enter_context(tc.tile_pool(name="v", bufs=6))
        rpool = ctx.enter_context(tc.tile_pool(name="r", bufs=4))
        apool = ctx.enter_context(tc.tile_pool(name="af", bufs=16))
        f8pool = ctx.enter_context(tc.tile_pool(name="f8", bufs=2))
        dpool = ctx.enter_context(tc.tile_pool(name="df", bufs=2))
        ypool = ctx.enter_context(tc.tile_pool(name="y", bufs=2))
        ppool = ctx.enter_context(tc.tile_pool(name="psum", bufs=2, space="PSUM"))

        # DMA priority order: the first matmul needs (a) the first weight
        # block and (b) features derived from x[sp0].  Land those first,
        # then interleave the rest so neither stream starves.
        xt = cpool.tile([INDIM, N_SHARD], FP32)
        wt = cpool.tile([INDIM, N_F16 * OUTDIM], FP16)
        w8t = cpool.tile([INDIM, 2, len(DR_G) * OUTDIM], FP8)
        btb = cpool.tile([INDIM, 18], FP32)
        bt = btb[:, 0:16]
        bias = btb[:, 16:18]
        # x chunk 0 first (it gates the whole feature chain), then the
        # ACT bias table, first weight block, and the rest interleaved.
        nc.sync.dma_start(xt[:, 0:512], xt_d[:, 0:512])
        nc.sync.dma_start(btb[:], btb_d[:])
        nc.sync.dma_start(wt[:, 0:2 * OUTDIM], w_d[:, 0:2 * OUTDIM])
        x_chunks = [(512, 1536), (1536, 2560), (2560, 3584), (3584, 4096)]
        w_slabs = [(2, 8), (8, 14), (14, 20), (20, 26)]
        for ci, ((xlo, xhi), (klo, khi)) in enumerate(zip(x_chunks, w_slabs)):
            nc.sync.dma_start(xt[:, xlo:xhi], xt_d[:, xlo:xhi])
            nc.sync.dma_start(wt[:, klo * OUTDIM:khi * OUTDIM],
                              w_d[:, klo * OUTDIM:khi * OUTDIM])
            if ci == 0:
                nc.sync.dma_start(w8t[:], w8_d[:])
        xts = [xt[:, SP_OFF[sp]:SP_OFF[sp + 1]] for sp in range(SP)]

        # view of yt for single-descriptor stores covering both row halves:
        # [r, h, c] -> yt[h*128 + r, c]
        yt_r = yt_d.rearrange("(h r) n -> r h n", h=2)

        def emit_evict(psums, sp, lo=0, hi=None):
            # split evictions across DVE (oh0) and ACT (oh1) so neither
            # queue eats both PSUM reads; store both halves with ONE DMA
            # descriptor (HWDGE descriptor processing is ~625ns, serial).
            # [lo,hi) selects a column chunk of the superpass.
            if hi is None:
                hi = SP_W[sp]
            osl = slice(SP_OFF[sp] + lo, SP_OFF[sp] + hi)
            y01 = ypool.tile([128, 2, 1024], FP16, tag="y01")
            nc.vector.tensor_scalar(y01[:, 0, lo:hi], psums[0][:, lo:hi],
                                    bias[:, 0:1], None,
                                    mybir.AluOpType.add)
            nc.scalar.activation(y01[:, 1, lo:hi], psums[1][:, lo:hi],
                                 mybir.ActivationFunctionType.Identity,
                                 bias=bias[:, 1:2], scale=1.0)
            nc.sync.dma_start(yt_r[:, :, osl], y01[:, :, lo:hi])

        def body():
            pending = None
            for sp in range(SP):
                xs = xts[sp]
                W = SP_W[sp]
                feats = {}

                # feature tiles are allocated at the max width (1024) so a
                # pool tag always recycles equal-size buffers; narrow
                # superpasses just use the first W columns.

                # -- range reduction + ACT passes for odd g --
                def reduce_g(g, j):
                    a, phat, c, b_s, b_c = _g_consts(g)
                    v = vpool.tile([INDIM, 1024], I32, tag="v")
                    nc.gpsimd.tensor_scalar(v[:, 0:W], xs, float(a), float(c),
                                            mybir.AluOpType.mult,
                                            mybir.AluOpType.add)
                    r = rpool.tile([INDIM, 1024], FP32, tag="r")
                    nc.vector.scalar_tensor_tensor(r[:, 0:W], v[:, 0:W],
                                                   float(-phat), xs,
                                                   mybir.AluOpType.mult,
                                                   mybir.AluOpType.add)
                    if g in DR_G:
                        # packed fp8 (sin, cos) pair for a DoubleRow block
                        f8 = f8pool.tile([INDIM, 2, 1024], FP8,
                                         tag=f"f8_{g}", name=f"f8_{g}")
                        nc.scalar.activation(f8[:, 0, 0:W], r[:, 0:W],
                                             mybir.ActivationFunctionType.Sin,
                                             bias=bt[:, 2 * j:2 * j + 1],
                                             scale=float(g))
                        nc.scalar.activation(f8[:, 1, 0:W], r[:, 0:W],
                                             mybir.ActivationFunctionType.Sin,
                                             bias=bt[:, 2 * j + 1:2 * j + 2],
                                             scale=float(g))
                        feats[("dr", g)] = f8
                        return
                    s = apool.tile([INDIM, 1024], FP16, tag="act")
                    nc.scalar.activation(s[:, 0:W], r[:, 0:W],
                                         mybir.ActivationFunctionType.Sin,
                                         bias=bt[:, 2 * j:2 * j + 1],
                                         scale=float(g))
                    cc = apool.tile([INDIM, 1024], FP16, tag="act")
                    nc.scalar.activation(cc[:, 0:W], r[:, 0:W],
                                         mybir.ActivationFunctionType.Sin,
                                         bias=bt[:, 2 * j + 1:2 * j + 2],
                                         scale=float(g))
                    feats[("s", g)] = s
                    feats[("c", g)] = cc

                def prod(name, m, in0, in1):
                    t = dpool.tile([INDIM, 1024], FP16, tag=f"{name}{m}")
                    nc.vector.tensor_tensor(t[:, 0:W], in0[:, 0:W],
                                            in1[:, 0:W],
                                            mybir.AluOpType.mult)
                    feats[(name, m)] = t
                    return t

                def truecos(m, vm, lam):
                    t = dpool.tile([INDIM, 1024], FP16, tag=f"C{2*m}")
                    nc.vector.tensor_scalar(t[:, 0:W], vm[:, 0:W],
                                            float(-lam), 1.0,
                                            mybir.AluOpType.mult,
                                            mybir.AluOpType.add)
                    return t

                # emission order = per-engine program order; keep DVE stream
                # interleaved so fmas feed ACT early and cascade flows.
                reduce_g(1, 0)
                reduce_g(3, 1)
                u1 = prod("u", 1, feats[("s", 1)], feats[("c", 1)])
                v1 = prod("v", 1, feats[("s", 1)], feats[("s", 1)])
                C2 = truecos(1, v1, LAMBDA[1])
                reduce_g(5, 2)
                u3 = prod("u", 3, feats[("s", 3)], feats[("c", 3)])
                v3 = prod("v", 3, feats[("s", 3)], feats[("s", 3)])
                C6 = truecos(3, v3, LAMBDA[3])
                reduce_g(7, 3)
                u5 = prod("u", 5, feats[("s", 5)], feats[("c", 5)])
                v5 = prod("v", 5, feats[("s", 5)], feats[("s", 5)])
                reduce_g(9, 4)
                u7 = prod("u", 7, feats[("s", 7)], feats[("c", 7)])
                v7 = prod("v", 7, feats[("s", 7)], feats[("s", 7)])
                reduce_g(11, 5)
                u2 = prod("u", 2, u1, C2)
                v2 = prod("v", 2, u1, u1)
                C4 = truecos(2, v2, LAMBDA[2])
                reduce_g(13, 6)
                u4 = prod("u", 4, u2, C4)
                v4 = prod("v", 4, u2, u2)
                C8 = truecos(4, v4, LAMBDA[4])
                reduce_g(15, 7)
                u6 = prod("u", 6, u3, C6)
                v6 = prod("v", 6, u3, u3)
                u8 = prod("u", 8, u4, C8)
                v8 = prod("v", 8, u4, u4)

                # previous superpass's eviction goes AFTER this superpass's
                # feature emission so ACT's program order is
                # [sp passes][sp+1 passes][evict sp] — eviction fires the
                # moment the PE frees the PSUM, without blocking features.
                if pending is not None:
                    emit_evict(*pending)

                # -- matmuls --
                n_kt = len(KT_ORDER)
                psum0 = ppool.tile([128, 1024], FP32, tag="p0")
                psum1 = ppool.tile([128, 1024], FP32, tag="p1")
                psums = [psum0, psum1]
                last = sp == SP - 1
                if last and pending is not None:
                    emit_evict(*pending)
                    pending = None
                for kt, fkey in enumerate(KT_ORDER):
                    f = feats[fkey]
                    st, stp = (kt == 0), (kt == n_kt - 1)
                    for oh in range(2):
                        if fkey[0] == "dr":
                            b = DR_G.index(fkey[1])
                            lhsT8 = w8t[:, :, (2 * b + oh) * 128:
                                        (2 * b + oh + 1) * 128]
                            for chi in range(W // CH):
                                nc.tensor.matmul(
                                    psums[oh][:, chi * CH:(chi + 1) * CH],
                                    lhsT8, f[:, :, chi * CH:(chi + 1) * CH],
                                    start=st, stop=stp,
                                    perf_mode=mybir.MatmulPerfMode.DoubleRow,
                                )
                            continue
                        kb = F16_IDX[fkey]
                        lhsT = wt[:, kb * OUTDIM + oh * 128:
                                  kb * OUTDIM + oh * 128 + 128]
                        for chi in range(W // CH):
                            nc.tensor.matmul(
                                psums[oh][:, chi * CH:(chi + 1) * CH],
                                lhsT, f[:, chi * CH:(chi + 1) * CH],
                                start=st, stop=stp,
                            )
                if not last:
                    pending = (psums, sp)
                else:
                    # evict per 512-column chunk so the first chunk's
                    # eviction + store start while the final matmuls of
                    # the second chunk are still draining.
                    for chi in range(W // CH):
                        emit_evict(psums, sp, lo=chi * CH, hi=(chi + 1) * CH)

        if reps == 1:
            body()
        else:
            with tc.For_i(0, reps, 1):
                body()

    nc.compile()
    _CACHED[key] = nc
    return nc


def _prep_inputs(x: np.ndarray, fouriercoeffs: np.ndarray, bias: np.ndarray):
    xt = np.ascontiguousarray(x.astype(np.float32, copy=False).T)  # (128, 32768)
    C0 = fouriercoeffs[0].astype(np.float32)   # (256, 128, 16) cos coeffs
    C1 = fouriercoeffs[1].astype(np.float32)   # sin coeffs

    # folded fp16 weight blocks: w_sb[i, kb*256 + col], col = output o
    w_sb = np.empty((INDIM, N_F16 * OUTDIM), np.float32)
    for (name, m), kb in F16_IDX.items():
        if name == "s":
            blk = C1[:, :, m - 1]                      # (o, i)
        elif name == "c":
            blk = C0[:, :, m - 1]
        elif name == "u":
            blk = KAPPA[m] * C1[:, :, 2 * m - 1]
        else:  # "v"
            blk = -LAMBDA[m] * C0[:, :, 2 * m - 1]
        w_sb[:, kb * OUTDIM:(kb + 1) * OUTDIM] = blk.T
    w_sb = w_sb.astype(np.float16)

    # fp8 DoubleRow weight pairs for DR_G: w8[i, j, (2b+oh)*128 + o'] with
    # j=0 matching the packed sin tile, j=1 the cos tile (unscaled e4m3:
    # subnormal step there ~= the normal-range step at |w|~0.02)
    import ml_dtypes
    w8 = np.empty((INDIM, 2, len(DR_G) * OUTDIM), np.float32)
    for b, g in enumerate(DR_G):
        for oh in range(2):
            cols = slice((2 * b + oh) * 128, (2 * b + oh + 1) * 128)
            w8[:, 0, cols] = C1[oh * 128:(oh + 1) * 128, :, g - 1].T
            w8[:, 1, cols] = C0[oh * 128:(oh + 1) * 128, :, g - 1].T
    w8 = w8.astype(ml_dtypes.float8_e4m3)

    # ACT bias table: 16 passes in order (b_s, b_c) per odd g
    bvals = np.empty(16, np.float32)
    for j, g in enumerate(ODD):
        _, _, _, b_s, b_c = _g_consts(g)
        bvals[2 * j] = b_s
        bvals[2 * j + 1] = b_c
    bt = np.tile(bvals[None, :], (INDIM, 1)).astype(np.float32)

    # folded output bias: bias + sum_i C0[o,i,2m-1] over even harmonics
    bias_fold = bias.reshape(-1).astype(np.float64).copy()
    for m in (1, 2, 3, 4, 5, 6, 7, 8):
        bias_fold += C0[:, :, 2 * m - 1].astype(np.float64).sum(axis=1)
    bias_sb = np.ascontiguousarray(
        bias_fold.astype(np.float32).reshape(2, 128).T)      # (128, 2)
    btb = np.ascontiguousarray(
        np.concatenate([bt, bias_sb], axis=1))               # (128, 18)
    return xt, w_sb, w8, btb


def kernel(x: np.ndarray, fouriercoeffs: np.ndarray, bias: np.ndarray,
           _trace: bool = False):
    x = np.asarray(x)
    fouriercoeffs = np.asarray(fouriercoeffs)
    bias = np.asarray(bias)
    orig_shape = x.shape
    x2 = x.reshape(-1, INDIM)
    assert x2.shape == (N_TOTAL, INDIM), x2.shape

    nc = _build()
    xt, w_sb, w8, btb = _prep_inputs(x2, fouriercoeffs, bias)
    in_maps = []
    for c in range(N_CORES):
        in_maps.append({
            "xt": np.ascontiguousarray(xt[:, c * N_SHARD:(c + 1) * N_SHARD]),
            "w": w_sb,
            "w8": w8,
            "btb": btb,
        })
    res = run_bass_kernel_spmd(nc, in_maps, list(range(N_CORES)),
                               trace=_trace)
    yt = np.concatenate([res.results[c]["yt"] for c in range(N_CORES)], axis=1)
    y = np.ascontiguousarray(yt.T).astype(np.float32)
    if _trace:
        kernel._last_result = res
    return y.reshape(*orig_shape[:-1], OUTDIM)



# revision 42
# speedup vs baseline: 1.0838x; 1.0012x over previous
"""KAN Fourier-linear kernel for 8 Trainium2 NeuronCores.

y[n,o] = sum_{i,g} C0[o,i,g]*cos(g*x[n,i]) + C1[o,i,g]*sin(g*x[n,i]) + bias[o]

Strategy (data-parallel over n, 4096 rows/core), double-angle cascade:
  - ACT (ScalarE) computes sin/cos only for odd g in {1,3,...,15} (16 Sin
    passes per superpass instead of 64), via the int-round range reduction:
      v   = int32(x*g/2pi + magic)        # gpsimd tensor_scalar
      r_g = x - v*(2pi/g)                 # DVE scalar_tensor_tensor (fp32)
      s_g = Sin(scale=g, bias=b_s)(r_g); c_g = Sin(scale=g, bias=b_c)(r_g)
  - Even harmonics come from 1-op DVE fp16 products with the scale factors
    folded into the weights host-side:
      u_m = s_m*c_m   covers sin(2m x) = kappa_m * u_m
      v_m = s_m*s_m   covers cos(2m x) = 1 - lambda_m * v_m
    (constants fold into the output bias). True-cos intermediates C_{2m} =
    1 - lambda*v_m (one tensor_scalar) extend the cascade to m in {2,4,6,8}.
  - Bulk precision fp16 (not bf16): the x128 weight folds amplify feature
    rounding error; fp16's 10-bit mantissa keeps that part at ~2e-3.
  - Mixed-precision PE: the (sin, cos) pairs for g in DR_G=(9,11,13) are
    emitted by ACT directly as packed fp8e4m3 tiles [128, 2, W] and
    contracted with fp8 DoubleRow matmuls (2 K-tiles per stream pass =
    2x MAC rate; measured ~291 ns per N=512 DR-MM == the fp16 rate).
    6 of 32 K-blocks in fp8 puts total max-rel error at 1.64e-2
    (deterministic, verified vs host sim; tolerance 2e-2).  4 pairs would
    be 2.03e-2 -> fails, and GPTQ-style compensated weight rounding can't
    help because the Fourier features are nearly orthogonal.
  - y.T tile = W.T @ F via PE, K-blocks accumulated in PSUM.
  - Superpass widths (512,512,1024x3): narrow first passes start the
    feature chain (and first matmul) early; wide bulk passes keep
    per-instruction overhead low (8x512 superpasses measured SLOWER:
    187us/rep vs 140 -- per-op overhead + narrow-MM cost dominate).
  - DMA order: x[0:512] first (gates the feature chain), merged bias
    table, first weight slab, then x/w interleaved; every dma_start costs
    a serialized ~625ns HWDGE descriptor slot, so fewer+ordered DMAs
    matter.  Output stores use one descriptor per superpass via a
    rearranged [r, h, c] view of yt covering both 128-row halves.
  - PSUM evicted split across DVE (rows 0:128) and ACT (128:256) with the
    fused per-partition bias add; the last superpass evicts per 512-col
    chunk so the store overlaps the final matmul drain.

Measured on 8xTRN2 (axon): steady-state 139.5 us/rep (baseline 183.3),
max rel err 1.64e-2.  PE floor at the measured 0.56 ns/col stream rate
is ~135 us; fp16-only variant was 147.9 us.
"""
import math
import numpy as np
from contextlib import ExitStack

import concourse.bass as bass
import concourse.mybir as mybir
import concourse.tile as tile
from concourse import bacc
from concourse.bass_utils import run_bass_kernel_spmd

N_CORES = 8
N_TOTAL = 32768
N_SHARD = N_TOTAL // N_CORES        # 4096 rows per core
INDIM = 128
OUTDIM = 256
GRID = 16
K_TOT = 2 * GRID * INDIM            # 4096
# superpass column widths: two narrow passes up front so the first
# feature chain (and hence the first matmul) starts early, wide passes
# for the bulk to keep per-instruction overhead low.
SP_W = (512, 512, 1024, 1024, 1024)
SP = len(SP_W)
SP_OFF = tuple(int(np.sum(SP_W[:i])) for i in range(SP + 1))
CH = 512                            # matmul moving chunk (PSUM bank limit)
TWO_PI = 2.0 * math.pi

FP32 = mybir.dt.float32
FP16 = mybir.dt.float16
FP8 = mybir.dt.float8e4
I32 = mybir.dt.int32

ODD = (1, 3, 5, 7, 9, 11, 13, 15)
# Harmonics whose (sin, cos) pair is packed into one fp8 tile and
# contracted with a single DoubleRow matmul (2 K-tiles per stream pass,
# 2x MAC rate).  4 of 32 K-blocks in fp8 keeps the extra quantization
# error at ~1.4e-2 max-rel (tolerance 2e-2); measured DR throughput is
# ~291 ns per N=512 DR-MM == the plain fp16 rate at twice the MACs.
DR_G = (9, 11, 13)
# kt consumption order: feature name per contraction block ("dr", g)
# entries consume a packed fp8 pair via one DoubleRow matmul.
# Interleaved so production (ACT for s/c, DVE for u/v) stays ahead of the PE.
KT_ORDER = [
    ("s", 1), ("c", 1), ("u", 1), ("v", 1),
    ("s", 3), ("c", 3), ("u", 3), ("v", 3),
    ("s", 5), ("c", 5), ("u", 5), ("v", 5),
    ("s", 7), ("c", 7), ("u", 7), ("v", 7),
    ("dr", 9), ("dr", 11), ("dr", 13),
    ("u", 2), ("v", 2), ("u", 4), ("v", 4), ("u", 6), ("v", 6),
    ("s", 15), ("c", 15), ("u", 8), ("v", 8),
]
# fp16 weight-block index for each non-dr entry, in order
F16_IDX = {}
for _e in KT_ORDER:
    if _e[0] != "dr":
        F16_IDX[_e] = len(F16_IDX)
N_F16 = len(F16_IDX)                # 28 fp16 K-blocks
KAPPA = {1: 2.0, 2: 4.0, 3: 2.0, 4: 8.0, 5: 2.0, 6: 4.0, 7: 2.0, 8: 16.0}
LAMBDA = {1: 2.0, 2: 8.0, 3: 2.0, 4: 32.0, 5: 2.0, 6: 8.0, 7: 2.0, 8: 128.0}


def _g_consts(g: int):
    a = np.float32(g / TWO_PI)
    phat = np.float32(TWO_PI / g)
    m = 2.0 ** math.ceil(math.log2(0.960 * g + 0.14))
    c = np.float32(m + 0.125)
    b_s = np.float32(m * g * float(phat))      # == 2pi*m up to fp32, matched to phat
    b_c = np.float32(float(b_s) + math.pi / 2.0)
    return a, phat, c, b_s, b_c


_CACHED = {}


def _build(reps: int = 1):
    key = ("nc", reps)
    if key in _CACHED:
        return _CACHED[key]
    nc = bacc.Bacc("TRN2", target_bir_lowering=False, debug=False,
                   num_devices=N_CORES)
    xt_d = nc.dram_tensor("xt", [INDIM, N_SHARD], FP32, kind="ExternalInput").ap()
    w_d = nc.dram_tensor("w", [INDIM, N_F16 * OUTDIM], FP16,
                         kind="ExternalInput").ap()
    w8_d = nc.dram_tensor("w8", [INDIM, 2, len(DR_G) * OUTDIM], FP8,
                          kind="ExternalInput").ap()
    btb_d = nc.dram_tensor("btb", [INDIM, 18], FP32, kind="ExternalInput").ap()
    yt_d = nc.dram_tensor("yt", [OUTDIM, N_SHARD], FP16, kind="ExternalOutput").ap()

    with tile.TileContext(nc) as tc, ExitStack() as ctx:
        cpool = ctx.enter_context(tc.tile_pool(name="const", bufs=1))
        vpool = ctx.enter_context(tc.tile_pool(name="v", bufs=6))
        rpool = ctx.enter_context(tc.tile_pool(name="r", bufs=4))
        apool = ctx.enter_context(tc.tile_pool(name="af", bufs=16))
        f8pool = ctx.enter_context(tc.tile_pool(name="f8", bufs=2))
        dpool = ctx.enter_context(tc.tile_pool(name="df", bufs=2))
        ypool = ctx.enter_context(tc.tile_pool(name="y", bufs=2))
        ppool = ctx.enter_context(tc.tile_pool(name="psum", bufs=2, space="PSUM"))

        # DMA priority order: the first matmul needs (a) the first weight
        # block and (b) features derived from x[sp0].  Land those first,
        # then interleave the rest so neither stream starves.
        xt = cpool.tile([INDIM, N_SHARD], FP32)
        wt = cpool.tile([INDIM, N_F16 * OUTDIM], FP16)
        w8t = cpool.tile([INDIM, 2, len(DR_G) * OUTDIM], FP8)
        btb = cpool.tile([INDIM, 18], FP32)
        bt = btb[:, 0:16]
        bias = btb[:, 16:18]
        # x chunk 0 first (it gates the whole feature chain), then the
        # ACT bias table, first weight block, and the rest interleaved.
        nc.sync.dma_start(xt[:, 0:512], xt_d[:, 0:512])
        nc.sync.dma_start(btb[:], btb_d[:])
        nc.sync.dma_start(wt[:, 0:2 * OUTDIM], w_d[:, 0:2 * OUTDIM])
        x_chunks = [(512, 1536), (1536, 2560), (2560, 3584), (3584, 4096)]
        w_slabs = [(2, 8), (8, 14), (14, 20), (20, 26)]
        for ci, ((xlo, xhi), (klo, khi)) in enumerate(zip(x_chunks, w_slabs)):
            nc.sync.dma_start(xt[:, xlo:xhi], xt_d[:, xlo:xhi])
            nc.sync.dma_start(wt[:, klo * OUTDIM:khi * OUTDIM],
                              w_d[:, klo * OUTDIM:khi * OUTDIM])
            if ci == 0:
                nc.sync.dma_start(w8t[:], w8_d[:])
        xts = [xt[:, SP_OFF[sp]:SP_OFF[sp + 1]] for sp in range(SP)]

        # view of yt for single-descriptor stores covering both row halves:
        # [r, h, c] -> yt[h*128 + r, c]
        yt_r = yt_d.rearrange("(h r) n -> r h n", h=2)

        def emit_evict(psums, sp, lo=0, hi=None):
            # split evictions across DVE (oh0) and ACT (oh1) so neither
            # queue eats both PSUM reads; store both halves with ONE DMA
            # descriptor (HWDGE descriptor processing is ~625ns, serial).
            # [lo,hi) selects a column chunk of the superpass.
            if hi is None:
                hi = SP_W[sp]
            osl = slice(SP_OFF[sp] + lo, SP_OFF[sp] + hi)
            y01 = ypool.tile([128, 2, 1024], FP16, tag="y01")
            nc.vector.tensor_scalar(y01[:, 0, lo:hi], psums[0][:, lo:hi],
                                    bias[:, 0:1], None,
                                    mybir.AluOpType.add)
            nc.scalar.activation(y01[:, 1, lo:hi], psums[1][:, lo:hi],
                                 mybir.ActivationFunctionType.Identity,
                                 bias=bias[:, 1:2], scale=1.0)
            nc.sync.dma_start(yt_r[:, :, osl], y01[:, :, lo:hi])

        def body():
            pending = None
            for sp in range(SP):
                xs = xts[sp]
                W = SP_W[sp]
                feats = {}

                # feature tiles are allocated at the max width (1024) so a
                # pool tag always recycles equal-size buffers; narrow
                # superpasses just use the first W columns.

                # -- range reduction + ACT passes for odd g --
                def reduce_g(g, j):
                    a, phat, c, b_s, b_c = _g_consts(g)
                    v = vpool.tile([INDIM, 1024], I32, tag="v")
                    nc.gpsimd.tensor_scalar(v[:, 0:W], xs, float(a), float(c),
                                            mybir.AluOpType.mult,
                                            mybir.AluOpType.add)
                    r = rpool.tile([INDIM, 1024], FP32, tag="r")
                    nc.vector.scalar_tensor_tensor(r[:, 0:W], v[:, 0:W],
                                                   float(-phat), xs,
                                                   mybir.AluOpType.mult,
                                                   mybir.AluOpType.add)
                    if g in DR_G:
                        # packed fp8 (sin, cos) pair for a DoubleRow block
                        f8 = f8pool.tile([INDIM, 2, 1024], FP8,
                                         tag=f"f8_{g}", name=f"f8_{g}")
                        nc.scalar.activation(f8[:, 0, 0:W], r[:, 0:W],
                                             mybir.ActivationFunctionType.Sin,
                                             bias=bt[:, 2 * j:2 * j + 1],
                                             scale=float(g))
                        nc.scalar.activation(f8[:, 1, 0:W], r[:, 0:W],
                                             mybir.ActivationFunctionType.Sin,
                                             bias=bt[:, 2 * j + 1:2 * j + 2],
                                             scale=float(g))
                        feats[("dr", g)] = f8
                        return
                    s = apool.tile([INDIM, 1024], FP16, tag="act")
                    nc.scalar.activation(s[:, 0:W], r[:, 0:W],
                                         mybir.ActivationFunctionType.Sin,
                                         bias=bt[:, 2 * j:2 * j + 1],
                                         scale=float(g))
                    cc = apool.tile([INDIM, 1024], FP16, tag="act")
                    nc.scalar.activation(cc[:, 0:W], r[:, 0:W],
                                         mybir.ActivationFunctionType.Sin,
                                         bias=bt[:, 2 * j + 1:2 * j + 2],
                                         scale=float(g))
                    feats[("s", g)] = s
                    feats[("c", g)] = cc

                def prod(name, m, in0, in1):
                    t = dpool.tile([INDIM, 1024], FP16, tag=f"{name}{m}")
                    nc.vector.tensor_tensor(t[:, 0:W], in0[:, 0:W],
                                            in1[:, 0:W],
                                            mybir.AluOpType.mult)
                    feats[(name, m)] = t
                    return t

                def truecos(m, vm, lam):
                    t = dpool.tile([INDIM, 1024], FP16, tag=f"C{2*m}")
                    nc.vector.tensor_scalar(t[:, 0:W], vm[:, 0:W],
                                            float(-lam), 1.0,
                                            mybir.AluOpType.mult,
                                            mybir.AluOpType.add)
                    return t

                # emission order = per-engine program order; keep DVE stream
                # interleaved so fmas feed ACT early and cascade flows.
                reduce_g(1, 0)
                reduce_g(3, 1)
                u1 = prod("u", 1, feats[("s", 1)], feats[("c", 1)])
                v1 = prod("v", 1, feats[("s", 1)], feats[("s", 1)])
                C2 = truecos(1, v1, LAMBDA[1])
                reduce_g(5, 2)
                u3 = prod("u", 3, feats[("s", 3)], feats[("c", 3)])
                v3 = prod("v", 3, feats[("s", 3)], feats[("s", 3)])
                C6 = truecos(3, v3, LAMBDA[3])
                reduce_g(7, 3)
                u5 = prod("u", 5, feats[("s", 5)], feats[("c", 5)])
                v5 = prod("v", 5, feats[("s", 5)], feats[("s", 5)])
                reduce_g(9, 4)
                u7 = prod("u", 7, feats[("s", 7)], feats[("c", 7)])
                v7 = prod("v", 7, feats[("s", 7)], feats[("s", 7)])
                reduce_g(11, 5)
                u2 = prod("u", 2, u1, C2)
                v2 = prod("v", 2, u1, u1)
                C4 = truecos(2, v2, LAMBDA[2])
                reduce_g(13, 6)
                u4 = prod("u", 4, u2, C4)
                v4 = prod("v", 4, u2, u2)
                C8 = truecos(4, v4, LAMBDA[4])
                reduce_g(15, 7)
                u6 = prod("u", 6, u3, C6)
                v6 = prod("v", 6, u3, u3)
                u8 = prod("u", 8, u4, C8)
                v8 = prod("v", 8, u4, u4)

                # previous superpass's eviction goes AFTER this superpass's
                # feature emission so ACT's program order is
                # [sp passes][sp+1 passes][evict sp] — eviction fires the
                # moment the PE frees the PSUM, without blocking features.
                if pending is not None:
                    emit_evict(*pending)

                # -- matmuls --
                n_kt = len(KT_ORDER)
                psum0 = ppool.tile([128, 1024], FP32, tag="p0")
                psum1 = ppool.tile([128, 1024], FP32, tag="p1")
                psums = [psum0, psum1]
                last = sp == SP - 1
                if last and pending is not None:
                    emit_evict(*pending)
                    pending = None
                for kt, fkey in enumerate(KT_ORDER):
                    f = feats[fkey]
                    st, stp = (kt == 0), (kt == n_kt - 1)
                    for oh in range(2):
                        if fkey[0] == "dr":
                            b = DR_G.index(fkey[1])
                            lhsT8 = w8t[:, :, (2 * b + oh) * 128:
                                        (2 * b + oh + 1) * 128]
                            for chi in range(W // CH):
                                nc.tensor.matmul(
                                    psums[oh][:, chi * CH:(chi + 1) * CH],
                                    lhsT8, f[:, :, chi * CH:(chi + 1) * CH],
                                    start=st, stop=stp,
                                    perf_mode=mybir.MatmulPerfMode.DoubleRow,
                                )
                            continue
                        kb = F16_IDX[fkey]
                        lhsT = wt[:, kb * OUTDIM + oh * 128:
                                  kb * OUTDIM + oh * 128 + 128]
                        for chi in range(W // CH):
                            nc.tensor.matmul(
                                psums[oh][:, chi * CH:(chi + 1) * CH],
                                lhsT, f[:, chi * CH:(chi + 1) * CH],
                                start=st, stop=stp,
                            )
                if not last:
                    pending = (psums, sp)
                else:
                    # evict per 512-column chunk so the first chunk's
                    # eviction + store start while the final matmuls of
                    # the second chunk are still draining.
                    for chi in range(W // CH):
                        emit_evict(psums, sp, lo=chi * CH, hi=(chi + 1) * CH)

        if reps == 1:
            body()
        else:
            with tc.For_i(0, reps, 1):
                body()

    nc.compile()
    _CACHED[key] = nc
    return nc


def _prep_inputs(x: np.ndarray, fouriercoeffs: np.ndarray, bias: np.ndarray):
    xt = np.ascontiguousarray(x.astype(np.float32, copy=False).T)  # (128, 32768)
    C0 = fouriercoeffs[0].astype(np.float32)   # (256, 128, 16) cos coeffs
    C1 = fouriercoeffs[1].astype(np.float32)   # sin coeffs

    # folded fp16 weight blocks: w_sb[i, kb*256 + col], col = output o
    w_sb = np.empty((INDIM, N_F16 * OUTDIM), np.float32)
    for (name, m), kb in F16_IDX.items():
        if name == "s":
            blk = C1[:, :, m - 1]                      # (o, i)
        elif name == "c":
            blk = C0[:, :, m - 1]
        elif name == "u":
            blk = KAPPA[m] * C1[:, :, 2 * m - 1]
        else:  # "v"
            blk = -LAMBDA[m] * C0[:, :, 2 * m - 1]
        w_sb[:, kb * OUTDIM:(kb + 1) * OUTDIM] = blk.T
    w_sb = w_sb.astype(np.float16)

    # fp8 DoubleRow weight pairs for DR_G: w8[i, j, (2b+oh)*128 + o'] with
    # j=0 matching the packed sin tile, j=1 the cos tile (unscaled e4m3:
    # subnormal step there ~= the normal-range step at |w|~0.02)
    import ml_dtypes
    w8 = np.empty((INDIM, 2, len(DR_G) * OUTDIM), np.float32)
    for b, g in enumerate(DR_G):
        for oh in range(2):
            cols = slice((2 * b + oh) * 128, (2 * b + oh + 1) * 128)
            w8[:, 0, cols] = C1[oh * 128:(oh + 1) * 128, :, g - 1].T
            w8[:, 1, cols] = C0[oh * 128:(oh + 1) * 128, :, g - 1].T
    w8 = w8.astype(ml_dtypes.float8_e4m3)

    # ACT bias table: 16 passes in order (b_s, b_c) per odd g
    bvals = np.empty(16, np.float32)
    for j, g in enumerate(ODD):
        _, _, _, b_s, b_c = _g_consts(g)
        bvals[2 * j] = b_s
        bvals[2 * j + 1] = b_c
    bt = np.tile(bvals[None, :], (INDIM, 1)).astype(np.float32)

    # folded output bias: bias + sum_i C0[o,i,2m-1] over even harmonics
    bias_fold = bias.reshape(-1).astype(np.float64).copy()
    for m in (1, 2, 3, 4, 5, 6, 7, 8):
        bias_fold += C0[:, :, 2 * m - 1].astype(np.float64).sum(axis=1)
    bias_sb = np.ascontiguousarray(
        bias_fold.astype(np.float32).reshape(2, 128).T)      # (128, 2)
    btb = np.ascontiguousarray(
        np.concatenate([bt, bias_sb], axis=1))               # (128, 18)
    return xt, w_sb, w8, btb


def kernel(x: np.ndarray, fouriercoeffs: np.ndarray, bias: np.ndarray,
           _trace: bool = False):
    x = np.asarray(x)
    fouriercoeffs = np.asarray(fouriercoeffs)
    bias = np.asarray(bias)
    orig_shape = x.shape
    x2 = x.reshape(-1, INDIM)
    assert x2.shape == (N_TOTAL, INDIM), x2.shape

    nc = _build()
    xt, w_sb, w8, btb = _prep_inputs(x2, fouriercoeffs, bias)
    in_maps = []
    for c in range(N_CORES):
        in_maps.append({
            "xt": np.ascontiguousarray(xt[:, c * N_SHARD:(c + 1) * N_SHARD]),
            "w": w_sb,
            "w8": w8,
            "btb": btb,
        })
    res = run_bass_kernel_spmd(nc, in_maps, list(range(N_CORES)),
                               trace=_trace)
    yt = np.concatenate([res.results[c]["yt"] for c in range(N_CORES)], axis=1)
    y = np.ascontiguousarray(yt.T).astype(np.float32)
    if _trace:
        kernel._last_result = res
    return y.reshape(*orig_shape[:-1], OUTDIM)

